# revision 18
# baseline (speedup 1.0000x reference)
"""Trainium2 Bass kernel for the AKT (attention-with-distance-decay) problem.

Reference math (per batch b, head h, dk=32, S=2048, E=256):
    qh, kh, vh = per-head projections of q,k,v
    s  = qh @ kh^T / sqrt(dk)                    (causal-masked)
    p  = softmax(s)                              (softmax #1)
    tail[j] = sum_{j'>j} p[j']                   (1 - cumsum)
    dist = sqrt(clip(tail * (i-j), 0))
    te   = clip(exp(-softplus(gamma_h) * dist), 1e-5, 1e5)
    attn = softmax(where(mask, s*te, -inf))      (softmax #2)
    out  = (attn @ vh)  -> concat heads -> @ Wo^T + bo

Sharding: 8 cores = (batch b = core//2) x (head-group g = core%2, 4 heads each).
Every core runs the identical graph (SPMD); per-core inputs differ.  Each core
emits a partial output (its 4 heads' contribution through Wo); the host adds
the two partials per batch plus bo.

Device-side structure per core:
  - host pre-transposes q/k/v to [E, S] so projections contract over e on
    the partition dim; Wq and bq are pre-scaled by 1/sqrt(dk).
  - qh^T, kh^T stored [128(4h x 32d), S]; vh stored [S, 4h, 33] bf16 with a
    ones column so the AV matmul also yields the softmax-#2 denominator.
  - causal q-block loop (128 queries, extent = (k+1) key-blocks); the
    diagonal block is masked by accumulating ident^T @ triu(-1e30) onto the
    QK PSUM scores.
  - softmax #1 skips the max-subtraction (scores are O(5), fp32 exp is safe);
    the key-axis cumsum is a REVERSED tensor_tensor_scan giving the exact
    suffix-sum (no 1-x cancellation); its col 0 is the softmax denominator.
  - te = exp(-sqrt(gamma^2 * tail * pos / sigma)): gamma^2/sigma ride the
    scalar slot of one scalar_tensor_tensor; sqrt+exp fused across 4 heads.
  - softmax #2: e2 = exp(s * clip(te)) directly (no max, masked lanes are
    exp(-1e30)=0); Sigma2 via the ones column; normalization folded into a
    per-partition tensor_scalar on the AV output.
  - e2 (bf16) transposed for AV by the DMA xbar (sync engine), not PE.
"""

import os
import sys

for _p in ("/opt/trn_rl_repo", "/root/.axon_site/_ro/trn_rl_repo"):
    if os.path.isdir(_p) and _p not in sys.path:
        sys.path.insert(0, _p)

import numpy as np

import concourse.bacc as bacc
import concourse.bass as bass
import concourse.mybir as mybir
from concourse.tile import TileContext

B, S, E, H = 4, 2048, 256, 8
DK = E // H          # 32
HG = 4               # heads per core
D = HG * DK          # 128, per-core projected width
NCORES = 8

FP = mybir.dt.float32
BF = mybir.dt.bfloat16
AF = mybir.ActivationFunctionType
OP = mybir.AluOpType
NEG = -1e30


def build_nc(s_len=S, qk_f32r=False):
    """Build the single-core SPMD graph.  s_len parametrizes the sequence
    length for small-scale simulation tests (must be a multiple of 128)."""
    nqb = s_len // 128           # number of 128-query blocks
    nech = E // 128              # e-chunks (2)

    nc = bacc.Bacc()
    qT = nc.declare_dram_parameter("qT", [E, s_len], FP, isOutput=False)
    kT = nc.declare_dram_parameter("kT", [E, s_len], FP, isOutput=False)
    vT = nc.declare_dram_parameter("vT", [E, s_len], FP, isOutput=False)
    wqT = nc.declare_dram_parameter("wqT", [E, D], FP, isOutput=False)
    wkT = nc.declare_dram_parameter("wkT", [E, D], FP, isOutput=False)
    wvT = nc.declare_dram_parameter("wvT", [E, D], FP, isOutput=False)
    woT = nc.declare_dram_parameter("woT", [D, E], FP, isOutput=False)
    bqs = nc.declare_dram_parameter("bqs", [64, 2], FP, isOutput=False)
    bks = nc.declare_dram_parameter("bks", [64, 2], FP, isOutput=False)
    bvrow = nc.declare_dram_parameter("bvrow", [1, D], FP, isOutput=False)
    gsq = nc.declare_dram_parameter("gsq", [128, HG], FP, isOutput=False)
    out_part = nc.declare_dram_parameter("out_part", [s_len, E], FP, isOutput=True)

    qk_dt = mybir.dt.float32r if qk_f32r else FP

    with TileContext(nc) as tc:
        with (
            tc.tile_pool(name="consts", bufs=1) as consts,
            tc.tile_pool(name="persist", bufs=1) as persist,
        ):
            # ---- constants ----
            ident_f = consts.tile([128, 128], FP)
            nc.vector.memset(ident_f[:], 1.0)
            nc.gpsimd.affine_select(out=ident_f[:], in_=ident_f[:],
                                    compare_op=OP.is_equal, fill=0.0,
                                    base=0, pattern=[[-1, 128]], channel_multiplier=1)
            ident_b = consts.tile([128, 128], BF)
            nc.vector.tensor_copy(out=ident_b[:], in_=ident_f[:])
            # strict upper triangle = NEG, else 0 (diagonal-block causal mask)
            triu_neg = consts.tile([128, 128], BF)
            nc.gpsimd.memset(triu_neg[:], 0.0)
            nc.gpsimd.affine_select(out=triu_neg[:], in_=triu_neg[:],
                                    compare_op=OP.is_ge, fill=NEG,
                                    base=0, pattern=[[-1, 128]], channel_multiplier=1)
            ones1 = consts.tile([1, 128], FP)
            nc.vector.memset(ones1[:], 1.0)

            gsq_sb = consts.tile([128, HG], FP)
            nc.sync.dma_start(out=gsq_sb[:], in_=gsq[:])
            bq_sb = consts.tile([64, 2], FP)
            nc.sync.dma_start(out=bq_sb[:], in_=bqs[:])
            bk_sb = consts.tile([64, 2], FP)
            nc.sync.dma_start(out=bk_sb[:], in_=bks[:])
            bv_sb = consts.tile([1, D], FP)
            nc.sync.dma_start(out=bv_sb[:], in_=bvrow[:])
            wo_sb = consts.tile([D, E], FP)
            nc.sync.dma_start(out=wo_sb[:], in_=woT[:])

            # ---- persistent activations ----
            # head h lives at partitions (h%2)*32..+32, free-block h//2
            # (PE operands may only start at partition 0/32/64)
            qhT = persist.tile([64, 2, s_len], BF)
            khT = persist.tile([64, 2, s_len], BF)
            vh1 = persist.tile([128, nqb, HG, 33], BF)  # [s-part, s-blk, h, 32d+1]
            nc.vector.memset(vh1[:, :, :, 32:33], 1.0)

            # ---- phase 0: projections ----
            with (
                tc.tile_pool(name="ph0", bufs=2) as ph0,
                tc.tile_pool(name="ph0w", bufs=1) as ph0w,
                tc.tile_pool(name="ph0ps", bufs=2, space="PSUM") as ph0ps,
            ):
                wq_sb = ph0w.tile([128, nech, D], FP)
                wk_sb = ph0w.tile([128, nech, D], FP)
                wv_sb = ph0w.tile([128, nech, D], FP)
                nc.sync.dma_start(out=wq_sb[:], in_=wqT.rearrange("(c p) d -> p c d", p=128))
                nc.sync.dma_start(out=wk_sb[:], in_=wkT.rearrange("(c p) d -> p c d", p=128))
                nc.sync.dma_start(out=wv_sb[:], in_=wvT.rearrange("(c p) d -> p c d", p=128))

                for name, src, wsb, bias, dst in (
                    ("q", qT, wq_sb, bq_sb, qhT),
                    ("k", kT, wk_sb, bk_sb, khT),
                ):
                    x_sb = ph0.tile([128, nech, s_len], FP, tag="x_in")
                    nc.sync.dma_start(out=x_sb[:],
                                      in_=src.rearrange("(c p) s -> p c s", p=128))
                    for dg in range(2):          # head-pairs (0,1) and (2,3)
                        for sc in range((s_len + 511) // 512):
                            s0, s1 = sc * 512, min((sc + 1) * 512, s_len)
                            ps = ph0ps.tile([64, 512], FP, tag=f"projps_{name}")
                            for c in range(nech):
                                nc.tensor.matmul(ps[:, 0:s1 - s0],
                                                 lhsT=wsb[:, c, dg * 64:(dg + 1) * 64],
                                                 rhs=x_sb[:, c, s0:s1],
                                                 start=(c == 0), stop=(c == nech - 1))
                            nc.scalar.activation(out=dst[:, dg, s0:s1],
                                                 in_=ps[:, 0:s1 - s0], func=AF.Identity,
                                                 bias=bias[:, dg:dg + 1])

                # vh: natural [s, d] orientation + bias row + bf16 cast
                xv_sb = ph0.tile([128, nech, s_len], FP, tag="x_in")
                nc.sync.dma_start(out=xv_sb[:],
                                  in_=vT.rearrange("(c p) s -> p c s", p=128))
                for sb in range(nqb):
                    ps = ph0ps.tile([128, 128], FP, tag="vps")
                    for c in range(nech):
                        nc.tensor.matmul(ps[:], lhsT=xv_sb[:, c, sb * 128:(sb + 1) * 128],
                                         rhs=wv_sb[:, c, :], start=(c == 0), stop=False)
                    nc.tensor.matmul(ps[:], lhsT=ones1[:], rhs=bv_sb[:],
                                     start=False, stop=True)
                    for h in range(HG):
                        nc.scalar.activation(out=vh1[:, sb, h, 0:32],
                                             in_=ps[:, h * 32:(h + 1) * 32], func=AF.Copy)

            # ---- attention loop ----
            with (
                tc.tile_pool(name="att", bufs=1) as att,
                tc.tile_pool(name="att2", bufs=2) as att2,
                tc.tile_pool(name="att3", bufs=3) as att3,
                tc.tile_pool(name="ps_sm", bufs=1, space="PSUM") as ps_sm,
                tc.tile_pool(name="ps_lg", bufs=1, space="PSUM") as ps_lg,
                tc.tile_pool(name="ps_av", bufs=1, space="PSUM") as ps_av,
                tc.tile_pool(name="ps_op", bufs=1, space="PSUM") as ps_op,
            ):
                # pair small and large extents so the two PSUM score buffers
                # (2 + 4 banks) let two tiles pipeline at once
                half = (nqb + 1) // 2
                pairs = [(lo, nqb - 1 - lo) for lo in range(half)]

                def attn_tile(kq, h, posk, concat):
                    nb = kq + 1            # causal extent in 128-key blocks
                    N = nb * 128
                    small = kq < half
                    pool = ps_sm if small else ps_lg
                    s_ps = pool.tile([128, half * 128 if small else nqb * 128],
                                     FP, tag="s_sm" if small else "s_lg")
                    # QK^T: chunks of <=512 moving columns
                    hp, hb = (h % 2) * 32, h // 2
                    nchunk = (N + 511) // 512
                    for c in range(nchunk):
                        c0, c1 = c * 512, min((c + 1) * 512, N)
                        nc.tensor.matmul(
                            s_ps[:, c0:c1],
                            lhsT=qhT[hp:hp + 32, hb, kq * 128:(kq + 1) * 128],
                            rhs=khT[hp:hp + 32, hb, c0:c1],
                            start=True, stop=True, skip_group_check=True)
                    # diagonal-block causal mask: += I^T @ triu(NEG)
                    nc.tensor.matmul(s_ps[:, N - 128:N], lhsT=ident_b[:],
                                     rhs=triu_neg[:], start=False, stop=True,
                                     skip_group_check=True)

                    # softmax #1 numerator (no max shift needed, |s| is small)
                    e = att3.tile([128, s_len], FP, tag="e")
                    nc.scalar.activation(out=e[:, :N], in_=s_ps[:, :N], func=AF.Exp)

                    # suffix sums: tail[j] = sum_{j'>=j} e[j'] (reversed scan);
                    # tail[0] is the softmax denominator, tail[j+1] the
                    # exclusive tail the decay term needs
                    tail = att3.tile([128, s_len + 2], FP, tag="tail")
                    nc.gpsimd.memset(tail[:, N:N + 1], 0.0)
                    nc.vector.tensor_tensor_scan(
                        out=tail[:, 0:N][:, ::-1], data0=e[:, 0:N][:, ::-1],
                        data1=e[:, 0:N][:, ::-1], initial=0.0,
                        op0=OP.add, op1=OP.bypass)

                    # rp = gamma_h^2 / sigma1
                    rp = att3.tile([128, 1], FP, tag="rp")
                    nc.vector.reciprocal(out=rp[:], in_=tail[:, 0:1])
                    nc.vector.tensor_scalar(out=rp[:], in0=rp[:],
                                            scalar1=gsq_sb[:, h:h + 1],
                                            scalar2=None, op0=OP.mult)
                    # tsq = gamma^2 * tail_excl * pos / sigma1
                    tsq = att2.tile([128, s_len], FP, tag="tsq")
                    nc.vector.scalar_tensor_tensor(
                        out=tsq[:, :N], in0=tail[:, 1:N + 1], scalar=rp[:],
                        in1=posk[:, :N], op0=OP.mult, op1=OP.mult)
                    # dist = sqrt(tsq) = exp(0.5*ln(tsq)) -- Ln+Exp share one
                    # ACT table set (Sqrt doesn't fit beside Exp);
                    # tsq=+0 -> ln=-inf -> dist=0 -> te=1 exactly
                    nc.scalar.activation(out=tsq[:, :N], in_=tsq[:, :N], func=AF.Ln)
                    nc.scalar.activation(out=tsq[:, :N], in_=tsq[:, :N],
                                         func=AF.Exp, scale=0.5)
                    nc.scalar.activation(out=tsq[:, :N], in_=tsq[:, :N],
                                         func=AF.Exp, scale=-1.0)

                    # s2 = max(te, 1e-5) * s   (masked lanes stay ~ -1e30)
                    s2 = att2.tile([128, s_len], FP, tag="s2")
                    nc.vector.scalar_tensor_tensor(
                        out=s2[:, :N], in0=tsq[:, :N], scalar=1e-5,
                        in1=s_ps[:, :N], op0=OP.max, op1=OP.mult)
                    # softmax #2 numerator
                    e2 = att2.tile([128, s_len], BF, tag="e2")
                    nc.scalar.activation(out=e2[:, :N], in_=s2[:, :N], func=AF.Exp)

                    e2t = att2.tile([128, nqb, 128], BF, tag="e2t")
                    nc.sync.dma_start_transpose(out=e2t[:, 0:nb, :], in_=e2[:, :N])
                    av = ps_av.tile([128, 64], FP, tag="av")
                    for c in range(nb):
                        nc.tensor.matmul(av[:, 0:33], lhsT=e2t[:, c, :],
                                         rhs=vh1[:, c, h, :],
                                         start=(c == 0), stop=(c == nb - 1))
                    rec2 = att3.tile([128, 1], FP, tag="rec2")
                    nc.vector.reciprocal(out=rec2[:], in_=av[:, 32:33])
                    nc.vector.tensor_scalar(
                        out=concat[:, h * 32:(h + 1) * 32], in0=av[:, 0:32],
                        scalar1=rec2[:], scalar2=None, op0=OP.mult)

                def out_proj(kq, concat):
                    trp = ps_op.tile([128, 128], FP, tag="trop")
                    nc.tensor.transpose(out=trp[:], in_=concat[:], identity=ident_f[:])
                    concatT = att2.tile([128, 128], FP, tag="concatT")
                    nc.scalar.activation(out=concatT[:], in_=trp[:], func=AF.Copy)
                    op = ps_op.tile([128, 256], FP, tag="trop")
                    nc.tensor.matmul(op[:], lhsT=concatT[:], rhs=wo_sb[:],
                                     start=True, stop=True)
                    ostg = att2.tile([128, 256], FP, tag="ostg")
                    nc.scalar.activation(out=ostg[:], in_=op[:], func=AF.Copy)
                    nc.sync.dma_start(out=out_part[kq * 128:(kq + 1) * 128, :],
                                      in_=ostg[:])

                for ksm, klg in pairs:
                    posk_s = att2.tile([128, half * 128], FP, tag="pos_s")
                    nc.gpsimd.iota(posk_s[:, :(ksm + 1) * 128],
                                   pattern=[[-1, (ksm + 1) * 128]], base=128 * ksm,
                                   channel_multiplier=1,
                                   allow_small_or_imprecise_dtypes=True)
                    nc.gpsimd.affine_select(
                        out=posk_s[:, ksm * 128:(ksm + 1) * 128],
                        in_=posk_s[:, ksm * 128:(ksm + 1) * 128],
                        compare_op=OP.is_ge, fill=0.0, base=0,
                        pattern=[[-1, 128]], channel_multiplier=1)
                    concat_s = att2.tile([128, 128], FP, tag="concat_s")
                    if klg != ksm:
                        posk_l = att2.tile([128, nqb * 128], FP, tag="pos_l")
                        nc.gpsimd.iota(posk_l[:, :(klg + 1) * 128],
                                       pattern=[[-1, (klg + 1) * 128]], base=128 * klg,
                                       channel_multiplier=1,
                                       allow_small_or_imprecise_dtypes=True)
                        nc.gpsimd.affine_select(
                            out=posk_l[:, klg * 128:(klg + 1) * 128],
                            in_=posk_l[:, klg * 128:(klg + 1) * 128],
                            compare_op=OP.is_ge, fill=0.0, base=0,
                            pattern=[[-1, 128]], channel_multiplier=1)
                        concat_l = att2.tile([128, 128], FP, tag="concat_l")
                    for h in range(HG):
                        attn_tile(ksm, h, posk_s, concat_s)
                        if klg != ksm:
                            attn_tile(klg, h, posk_l, concat_l)
                    out_proj(ksm, concat_s)
                    if klg != ksm:
                        out_proj(klg, concat_l)
    return nc


# ---------------------------------------------------------------------------
# host side
# ---------------------------------------------------------------------------

def _softplus(x):
    return np.logaddexp(0.0, x)


def _make_in_maps(q, k, v, Wq, bq, Wk, bk, Wv, bv, Wo, gammas, s_len=S):
    scale = 1.0 / np.sqrt(np.float32(DK))
    g = -_softplus(gammas.reshape(H).astype(np.float64)).astype(np.float32)
    in_maps = []
    for core in range(NCORES):
        b, grp = core // 2, core % 2
        hsel = slice(grp * HG * DK, (grp + 1) * HG * DK)   # rows of W, dims of proj
        gam = g[grp * HG:(grp + 1) * HG]
        in_maps.append({
            "qT": np.ascontiguousarray(q[b].T.astype(np.float32)),
            "kT": np.ascontiguousarray(k[b].T.astype(np.float32)),
            "vT": np.ascontiguousarray(v[b].T.astype(np.float32)),
            "wqT": np.ascontiguousarray((Wq[hsel, :] * scale).T.astype(np.float32)),
            "wkT": np.ascontiguousarray(Wk[hsel, :].T.astype(np.float32)),
            "wvT": np.ascontiguousarray(Wv[hsel, :].T.astype(np.float32)),
            "woT": np.ascontiguousarray(Wo[:, hsel].T.astype(np.float32)),
            "bqs": np.ascontiguousarray(
                (bq[hsel] * scale).astype(np.float32).reshape(2, 64).T),
            "bks": np.ascontiguousarray(
                bk[hsel].astype(np.float32).reshape(2, 64).T),
            "bvrow": bv[hsel].astype(np.float32).reshape(1, D),
            "gsq": np.broadcast_to((gam * gam).astype(np.float32), (128, HG)).copy(),
        })
    return in_maps


_NC_CACHE = {}


def _get_nc(s_len=S):
    if s_len not in _NC_CACHE:
        nc = build_nc(s_len)
        nc.finalize()      # Bacc pipeline: wait splitting, reg alloc, DCE
        _NC_CACHE[s_len] = nc
    return _NC_CACHE[s_len]


def kernel(q, k, v, mask, Wq, bq, Wk, bk, Wv, bv, Wo, bo, gammas):
    """Full-input, full-output entry point.  `mask` is the causal mask the
    reference builds; the kernel hardcodes causality."""
    from concourse.bass_utils import run_bass_kernel_spmd

    q, k, v = (np.asarray(a, np.float32) for a in (q, k, v))
    in_maps = _make_in_maps(q, k, v, np.asarray(Wq), np.asarray(bq),
                            np.asarray(Wk), np.asarray(bk), np.asarray(Wv),
                            np.asarray(bv), np.asarray(Wo),
                            np.asarray(gammas))
    nc = _get_nc(S)
    res = run_bass_kernel_spmd(nc, in_maps, core_ids=list(range(NCORES)))
    parts = [res.results[c]["out_part"] for c in range(NCORES)]
    out = np.empty((B, S, E), np.float32)
    bo = np.asarray(bo, np.float32)
    for b in range(B):
        out[b] = parts[2 * b] + parts[2 * b + 1] + bo[None, :]
    return out


# revision 20
# speedup vs baseline: 1.1419x; 1.1419x over previous
"""Trainium2 Bass kernel for the AKT (attention-with-distance-decay) problem.

Reference math (per batch b, head h, dk=32, S=2048, E=256):
    qh, kh, vh = per-head projections of q,k,v
    s  = qh @ kh^T / sqrt(dk)                    (causal-masked)
    p  = softmax(s)                              (softmax #1)
    tail[j] = sum_{j'>j} p[j']                   (1 - cumsum)
    dist = sqrt(clip(tail * (i-j), 0))
    te   = clip(exp(-softplus(gamma_h) * dist), 1e-5, 1e5)
    attn = softmax(where(mask, s*te, -inf))      (softmax #2)
    out  = (attn @ vh)  -> concat heads -> @ Wo^T + bo

Sharding: 8 cores = (batch b = core//2) x (head-group g = core%2, 4 heads each).
Every core runs the identical graph (SPMD); per-core inputs differ.  Each core
emits a partial output (its 4 heads' contribution through Wo); the host adds
the two partials per batch plus bo.

Device-side structure per core:
  - host pre-transposes q/k/v to [E, S] so projections contract over e on
    the partition dim; Wq and bq are pre-scaled by 1/sqrt(dk).
  - qh^T, kh^T stored [128(4h x 32d), S]; vh stored [S, 4h, 33] bf16 with a
    ones column so the AV matmul also yields the softmax-#2 denominator.
  - causal q-block loop (128 queries, extent = (k+1) key-blocks); the
    diagonal block is masked by accumulating ident^T @ triu(-1e30) onto the
    QK PSUM scores.
  - softmax #1 skips the max-subtraction (scores are O(5), fp32 exp is safe);
    the key-axis cumsum is a REVERSED tensor_tensor_scan giving the exact
    suffix-sum (no 1-x cancellation); its col 0 is the softmax denominator.
  - te = exp(-sqrt(gamma^2 * tail * pos / sigma)): gamma^2/sigma ride the
    scalar slot of one scalar_tensor_tensor; sqrt+exp fused across 4 heads.
  - softmax #2: e2 = exp(s * clip(te)) directly (no max, masked lanes are
    exp(-1e30)=0); Sigma2 via the ones column; normalization folded into a
    per-partition tensor_scalar on the AV output.
  - e2 (bf16) transposed for AV by the DMA xbar (sync engine), not PE.
"""

import os
import sys

for _p in ("/opt/trn_rl_repo", "/root/.axon_site/_ro/trn_rl_repo"):
    if os.path.isdir(_p) and _p not in sys.path:
        sys.path.insert(0, _p)

import numpy as np

import concourse.bacc as bacc
import concourse.bass as bass
import concourse.mybir as mybir
from concourse.tile import TileContext

B, S, E, H = 4, 2048, 256, 8
DK = E // H          # 32
HG = 4               # heads per core
D = HG * DK          # 128, per-core projected width
NCORES = 8

FP = mybir.dt.float32
BF = mybir.dt.bfloat16
AF = mybir.ActivationFunctionType
OP = mybir.AluOpType
NEG = -1e30


class _AktBacc(bacc.Bacc):
    """Bacc whose activation-table placement only considers the one set
    covering every ACT function this kernel uses (Exp, Ln, Identity, Copy).
    The default first-match policy alternates exp_and_others with a
    Ln-capable set, reloading the 2.7us ACT tables per tile."""

    _ACT_SET = "natural_log_exp_and_others"

    def insert_act_table_loads(self):
        import concourse.mybir as _mb
        from concourse.hw_specs import get_activation_tables
        has_activation = any(
            isinstance(i, _mb.InstActivation)
            for b in self.main_func.blocks
            for i in b.instructions
        )
        if not has_activation:
            return
        # positions must stay canonical (act_func_set_id indexes this list)
        tables = [
            (nm, fs if nm == self._ACT_SET else set())
            for nm, fs in get_activation_tables(self.m.arch).items()
        ]
        import bass_rust as _br
        _br.insert_act_table_loads(self, tables)


def build_nc(s_len=S, qk_f32r=False):
    """Build the single-core SPMD graph.  s_len parametrizes the sequence
    length for small-scale simulation tests (must be a multiple of 128)."""
    nqb = s_len // 128           # number of 128-query blocks
    nech = E // 128              # e-chunks (2)

    nc = _AktBacc()
    qT = nc.declare_dram_parameter("qT", [E, s_len], FP, isOutput=False)
    kT = nc.declare_dram_parameter("kT", [E, s_len], FP, isOutput=False)
    vT = nc.declare_dram_parameter("vT", [E, s_len], FP, isOutput=False)
    wqT = nc.declare_dram_parameter("wqT", [E, D], FP, isOutput=False)
    wkT = nc.declare_dram_parameter("wkT", [E, D], FP, isOutput=False)
    wvT = nc.declare_dram_parameter("wvT", [E, D], FP, isOutput=False)
    woT = nc.declare_dram_parameter("woT", [D, E], FP, isOutput=False)
    bqs = nc.declare_dram_parameter("bqs", [64, 2], FP, isOutput=False)
    bks = nc.declare_dram_parameter("bks", [64, 2], FP, isOutput=False)
    bvrow = nc.declare_dram_parameter("bvrow", [1, D], FP, isOutput=False)
    gsq = nc.declare_dram_parameter("gsq", [128, HG], FP, isOutput=False)
    out_part = nc.declare_dram_parameter("out_part", [s_len, E], FP, isOutput=True)

    qk_dt = mybir.dt.float32r if qk_f32r else FP

    with TileContext(nc) as tc:
        with (
            tc.tile_pool(name="consts", bufs=1) as consts,
            tc.tile_pool(name="persist", bufs=1) as persist,
        ):
            # ---- constants ----
            ident_f = consts.tile([128, 128], FP)
            nc.vector.memset(ident_f[:], 1.0)
            nc.gpsimd.affine_select(out=ident_f[:], in_=ident_f[:],
                                    compare_op=OP.is_equal, fill=0.0,
                                    base=0, pattern=[[-1, 128]], channel_multiplier=1)
            ident_b = consts.tile([128, 128], BF)
            nc.vector.tensor_copy(out=ident_b[:], in_=ident_f[:])
            # strict upper triangle = NEG, else 0 (diagonal-block causal mask)
            triu_neg = consts.tile([128, 128], BF)
            nc.gpsimd.memset(triu_neg[:], 0.0)
            nc.gpsimd.affine_select(out=triu_neg[:], in_=triu_neg[:],
                                    compare_op=OP.is_ge, fill=NEG,
                                    base=0, pattern=[[-1, 128]], channel_multiplier=1)
            ones1 = consts.tile([1, 128], FP)
            nc.vector.memset(ones1[:], 1.0)

            gsq_sb = consts.tile([128, HG], FP)
            nc.sync.dma_start(out=gsq_sb[:], in_=gsq[:])
            bq_sb = consts.tile([64, 2], FP)
            nc.sync.dma_start(out=bq_sb[:], in_=bqs[:])
            bk_sb = consts.tile([64, 2], FP)
            nc.sync.dma_start(out=bk_sb[:], in_=bks[:])
            bv_sb = consts.tile([1, D], FP)
            nc.sync.dma_start(out=bv_sb[:], in_=bvrow[:])
            wo_sb = consts.tile([D, E], FP)
            nc.sync.dma_start(out=wo_sb[:], in_=woT[:])

            # ---- persistent activations ----
            # head h lives at partitions (h%2)*32..+32, free-block h//2
            # (PE operands may only start at partition 0/32/64)
            qhT = persist.tile([64, 2, s_len], BF)
            khT = persist.tile([64, 2, s_len], BF)
            vh1 = persist.tile([128, nqb, HG, 33], BF)  # [s-part, s-blk, h, 32d+1]
            nc.vector.memset(vh1[:, :, :, 32:33], 1.0)

            # ---- phase 0: projections ----
            with (
                tc.tile_pool(name="ph0", bufs=2) as ph0,
                tc.tile_pool(name="ph0w", bufs=1) as ph0w,
                tc.tile_pool(name="ph0ps", bufs=2, space="PSUM") as ph0ps,
            ):
                wq_sb = ph0w.tile([128, nech, D], FP)
                wk_sb = ph0w.tile([128, nech, D], FP)
                wv_sb = ph0w.tile([128, nech, D], FP)
                nc.sync.dma_start(out=wq_sb[:], in_=wqT.rearrange("(c p) d -> p c d", p=128))
                nc.sync.dma_start(out=wk_sb[:], in_=wkT.rearrange("(c p) d -> p c d", p=128))
                nc.sync.dma_start(out=wv_sb[:], in_=wvT.rearrange("(c p) d -> p c d", p=128))

                for name, src, wsb, bias, dst in (
                    ("q", qT, wq_sb, bq_sb, qhT),
                    ("k", kT, wk_sb, bk_sb, khT),
                ):
                    x_sb = ph0.tile([128, nech, s_len], FP, tag="x_in")
                    nc.sync.dma_start(out=x_sb[:],
                                      in_=src.rearrange("(c p) s -> p c s", p=128))
                    for dg in range(2):          # head-pairs (0,1) and (2,3)
                        for sc in range((s_len + 511) // 512):
                            s0, s1 = sc * 512, min((sc + 1) * 512, s_len)
                            ps = ph0ps.tile([64, 512], FP, tag=f"projps_{name}")
                            for c in range(nech):
                                nc.tensor.matmul(ps[:, 0:s1 - s0],
                                                 lhsT=wsb[:, c, dg * 64:(dg + 1) * 64],
                                                 rhs=x_sb[:, c, s0:s1],
                                                 start=(c == 0), stop=(c == nech - 1))
                            nc.scalar.activation(out=dst[:, dg, s0:s1],
                                                 in_=ps[:, 0:s1 - s0], func=AF.Identity,
                                                 bias=bias[:, dg:dg + 1])

                # vh: natural [s, d] orientation + bias row + bf16 cast
                xv_sb = ph0.tile([128, nech, s_len], FP, tag="x_in")
                nc.sync.dma_start(out=xv_sb[:],
                                  in_=vT.rearrange("(c p) s -> p c s", p=128))
                for sb in range(nqb):
                    ps = ph0ps.tile([128, 128], FP, tag="vps")
                    for c in range(nech):
                        nc.tensor.matmul(ps[:], lhsT=xv_sb[:, c, sb * 128:(sb + 1) * 128],
                                         rhs=wv_sb[:, c, :], start=(c == 0), stop=False)
                    nc.tensor.matmul(ps[:], lhsT=ones1[:], rhs=bv_sb[:],
                                     start=False, stop=True)
                    for h in range(HG):
                        nc.scalar.activation(out=vh1[:, sb, h, 0:32],
                                             in_=ps[:, h * 32:(h + 1) * 32], func=AF.Copy)

            # ---- attention loop ----
            with (
                tc.tile_pool(name="att", bufs=1) as att,
                tc.tile_pool(name="att2", bufs=2) as att2,
                tc.tile_pool(name="att3", bufs=3) as att3,
                tc.tile_pool(name="ps_sm", bufs=1, space="PSUM") as ps_sm,
                tc.tile_pool(name="ps_lg", bufs=1, space="PSUM") as ps_lg,
                tc.tile_pool(name="ps_av", bufs=1, space="PSUM") as ps_av,
                tc.tile_pool(name="ps_op", bufs=1, space="PSUM") as ps_op,
            ):
                # pair small and large extents so the two PSUM score buffers
                # (2 + 4 banks) let two tiles pipeline at once
                half = (nqb + 1) // 2
                pairs = [(lo, nqb - 1 - lo) for lo in range(half)]

                def attn_tile(kq, h, posk, concat):
                    nb = kq + 1            # causal extent in 128-key blocks
                    N = nb * 128
                    small = kq < half
                    pool = ps_sm if small else ps_lg
                    s_ps = pool.tile([128, half * 128 if small else nqb * 128],
                                     FP, tag="s_sm" if small else "s_lg")
                    # QK^T: chunks of <=512 moving columns
                    hp, hb = (h % 2) * 32, h // 2
                    nchunk = (N + 511) // 512
                    for c in range(nchunk):
                        c0, c1 = c * 512, min((c + 1) * 512, N)
                        nc.tensor.matmul(
                            s_ps[:, c0:c1],
                            lhsT=qhT[hp:hp + 32, hb, kq * 128:(kq + 1) * 128],
                            rhs=khT[hp:hp + 32, hb, c0:c1],
                            start=True, stop=True, skip_group_check=True)
                    # diagonal-block causal mask: += I^T @ triu(NEG)
                    nc.tensor.matmul(s_ps[:, N - 128:N], lhsT=ident_b[:],
                                     rhs=triu_neg[:], start=False, stop=True,
                                     skip_group_check=True)

                    # softmax #1 numerator (no max shift needed, |s| is small)
                    e = att3.tile([128, s_len], FP, tag="e")
                    nc.scalar.activation(out=e[:, :N], in_=s_ps[:, :N], func=AF.Exp)

                    # suffix sums: tail[j] = sum_{j'>=j} e[j'] (reversed scan);
                    # tail[0] is the softmax denominator, tail[j+1] the
                    # exclusive tail the decay term needs
                    tail = att3.tile([128, s_len + 2], FP, tag="tail")
                    nc.gpsimd.memset(tail[:, N:N + 1], 0.0)
                    nc.vector.tensor_tensor_scan(
                        out=tail[:, 0:N][:, ::-1], data0=e[:, 0:N][:, ::-1],
                        data1=e[:, 0:N][:, ::-1], initial=0.0,
                        op0=OP.add, op1=OP.bypass)

                    # rp = gamma_h^2 / sigma1
                    rp = att3.tile([128, 1], FP, tag="rp")
                    nc.vector.reciprocal(out=rp[:], in_=tail[:, 0:1])
                    nc.vector.tensor_scalar(out=rp[:], in0=rp[:],
                                            scalar1=gsq_sb[:, h:h + 1],
                                            scalar2=None, op0=OP.mult)
                    # tsq = gamma^2 * tail_excl * pos / sigma1
                    tsq = att2.tile([128, s_len], FP, tag="tsq")
                    nc.vector.scalar_tensor_tensor(
                        out=tsq[:, :N], in0=tail[:, 1:N + 1], scalar=rp[:],
                        in1=posk[:, :N], op0=OP.mult, op1=OP.mult)
                    # dist = sqrt(tsq) = exp(0.5*ln(tsq)) -- Ln+Exp share one
                    # ACT table set (Sqrt doesn't fit beside Exp);
                    # tsq=+0 -> ln=-inf -> dist=0 -> te=1 exactly
                    nc.scalar.activation(out=tsq[:, :N], in_=tsq[:, :N], func=AF.Ln)
                    nc.scalar.activation(out=tsq[:, :N], in_=tsq[:, :N],
                                         func=AF.Exp, scale=0.5)
                    nc.scalar.activation(out=tsq[:, :N], in_=tsq[:, :N],
                                         func=AF.Exp, scale=-1.0)

                    # s2 = max(te, 1e-5) * s   (masked lanes stay ~ -1e30)
                    s2 = att2.tile([128, s_len], FP, tag="s2")
                    nc.vector.scalar_tensor_tensor(
                        out=s2[:, :N], in0=tsq[:, :N], scalar=1e-5,
                        in1=s_ps[:, :N], op0=OP.max, op1=OP.mult)
                    # softmax #2 numerator
                    e2 = att2.tile([128, s_len], BF, tag="e2")
                    nc.scalar.activation(out=e2[:, :N], in_=s2[:, :N], func=AF.Exp)

                    e2t = att2.tile([128, nqb, 128], BF, tag="e2t")
                    nc.sync.dma_start_transpose(out=e2t[:, 0:nb, :], in_=e2[:, :N])
                    av = ps_av.tile([128, 64], FP, tag="av")
                    for c in range(nb):
                        nc.tensor.matmul(av[:, 0:33], lhsT=e2t[:, c, :],
                                         rhs=vh1[:, c, h, :],
                                         start=(c == 0), stop=(c == nb - 1))
                    rec2 = att3.tile([128, 1], FP, tag="rec2")
                    nc.vector.reciprocal(out=rec2[:], in_=av[:, 32:33])
                    nc.vector.tensor_scalar(
                        out=concat[:, h * 32:(h + 1) * 32], in0=av[:, 0:32],
                        scalar1=rec2[:], scalar2=None, op0=OP.mult)

                def out_proj(kq, concat):
                    trp = ps_op.tile([128, 128], FP, tag="trop")
                    nc.tensor.transpose(out=trp[:], in_=concat[:], identity=ident_f[:])
                    concatT = att2.tile([128, 128], FP, tag="concatT")
                    nc.scalar.activation(out=concatT[:], in_=trp[:], func=AF.Copy)
                    op = ps_op.tile([128, 256], FP, tag="trop")
                    nc.tensor.matmul(op[:], lhsT=concatT[:], rhs=wo_sb[:],
                                     start=True, stop=True)
                    ostg = att2.tile([128, 256], FP, tag="ostg")
                    nc.scalar.activation(out=ostg[:], in_=op[:], func=AF.Copy)
                    nc.sync.dma_start(out=out_part[kq * 128:(kq + 1) * 128, :],
                                      in_=ostg[:])

                for ksm, klg in pairs:
                    posk_s = att2.tile([128, half * 128], FP, tag="pos_s")
                    nc.gpsimd.iota(posk_s[:, :(ksm + 1) * 128],
                                   pattern=[[-1, (ksm + 1) * 128]], base=128 * ksm,
                                   channel_multiplier=1,
                                   allow_small_or_imprecise_dtypes=True)
                    nc.gpsimd.affine_select(
                        out=posk_s[:, ksm * 128:(ksm + 1) * 128],
                        in_=posk_s[:, ksm * 128:(ksm + 1) * 128],
                        compare_op=OP.is_ge, fill=0.0, base=0,
                        pattern=[[-1, 128]], channel_multiplier=1)
                    concat_s = att2.tile([128, 128], FP, tag="concat_s")
                    if klg != ksm:
                        posk_l = att2.tile([128, nqb * 128], FP, tag="pos_l")
                        nc.gpsimd.iota(posk_l[:, :(klg + 1) * 128],
                                       pattern=[[-1, (klg + 1) * 128]], base=128 * klg,
                                       channel_multiplier=1,
                                       allow_small_or_imprecise_dtypes=True)
                        nc.gpsimd.affine_select(
                            out=posk_l[:, klg * 128:(klg + 1) * 128],
                            in_=posk_l[:, klg * 128:(klg + 1) * 128],
                            compare_op=OP.is_ge, fill=0.0, base=0,
                            pattern=[[-1, 128]], channel_multiplier=1)
                        concat_l = att2.tile([128, 128], FP, tag="concat_l")
                    for h in range(HG):
                        attn_tile(ksm, h, posk_s, concat_s)
                        if klg != ksm:
                            attn_tile(klg, h, posk_l, concat_l)
                    out_proj(ksm, concat_s)
                    if klg != ksm:
                        out_proj(klg, concat_l)
    return nc


# ---------------------------------------------------------------------------
# host side
# ---------------------------------------------------------------------------

def _softplus(x):
    return np.logaddexp(0.0, x)


def _make_in_maps(q, k, v, Wq, bq, Wk, bk, Wv, bv, Wo, gammas, s_len=S):
    scale = 1.0 / np.sqrt(np.float32(DK))
    g = -_softplus(gammas.reshape(H).astype(np.float64)).astype(np.float32)
    in_maps = []
    for core in range(NCORES):
        b, grp = core // 2, core % 2
        hsel = slice(grp * HG * DK, (grp + 1) * HG * DK)   # rows of W, dims of proj
        gam = g[grp * HG:(grp + 1) * HG]
        in_maps.append({
            "qT": np.ascontiguousarray(q[b].T.astype(np.float32)),
            "kT": np.ascontiguousarray(k[b].T.astype(np.float32)),
            "vT": np.ascontiguousarray(v[b].T.astype(np.float32)),
            "wqT": np.ascontiguousarray((Wq[hsel, :] * scale).T.astype(np.float32)),
            "wkT": np.ascontiguousarray(Wk[hsel, :].T.astype(np.float32)),
            "wvT": np.ascontiguousarray(Wv[hsel, :].T.astype(np.float32)),
            "woT": np.ascontiguousarray(Wo[:, hsel].T.astype(np.float32)),
            "bqs": np.ascontiguousarray(
                (bq[hsel] * scale).astype(np.float32).reshape(2, 64).T),
            "bks": np.ascontiguousarray(
                bk[hsel].astype(np.float32).reshape(2, 64).T),
            "bvrow": bv[hsel].astype(np.float32).reshape(1, D),
            "gsq": np.broadcast_to((gam * gam).astype(np.float32), (128, HG)).copy(),
        })
    return in_maps


_NC_CACHE = {}


def _get_nc(s_len=S):
    if s_len not in _NC_CACHE:
        nc = build_nc(s_len)
        nc.finalize()      # Bacc pipeline: wait splitting, reg alloc, DCE
        _NC_CACHE[s_len] = nc
    return _NC_CACHE[s_len]


def kernel(q, k, v, mask, Wq, bq, Wk, bk, Wv, bv, Wo, bo, gammas):
    """Full-input, full-output entry point.  `mask` is the causal mask the
    reference builds; the kernel hardcodes causality."""
    from concourse.bass_utils import run_bass_kernel_spmd

    q, k, v = (np.asarray(a, np.float32) for a in (q, k, v))
    in_maps = _make_in_maps(q, k, v, np.asarray(Wq), np.asarray(bq),
                            np.asarray(Wk), np.asarray(bk), np.asarray(Wv),
                            np.asarray(bv), np.asarray(Wo),
                            np.asarray(gammas))
    nc = _get_nc(S)
    res = run_bass_kernel_spmd(nc, in_maps, core_ids=list(range(NCORES)))
    parts = [res.results[c]["out_part"] for c in range(NCORES)]
    out = np.empty((B, S, E), np.float32)
    bo = np.asarray(bo, np.float32)
    for b in range(B):
        out[b] = parts[2 * b] + parts[2 * b + 1] + bo[None, :]
    return out


# revision 21
# speedup vs baseline: 1.1598x; 1.0157x over previous
"""Trainium2 Bass kernel for the AKT (attention-with-distance-decay) problem.

Reference math (per batch b, head h, dk=32, S=2048, E=256):
    qh, kh, vh = per-head projections of q,k,v
    s  = qh @ kh^T / sqrt(dk)                    (causal-masked)
    p  = softmax(s)                              (softmax #1)
    tail[j] = sum_{j'>j} p[j']                   (1 - cumsum)
    dist = sqrt(clip(tail * (i-j), 0))
    te   = clip(exp(-softplus(gamma_h) * dist), 1e-5, 1e5)
    attn = softmax(where(mask, s*te, -inf))      (softmax #2)
    out  = (attn @ vh)  -> concat heads -> @ Wo^T + bo

Sharding: 8 cores = (batch b = core//2) x (head-group g = core%2, 4 heads each).
Every core runs the identical graph (SPMD); per-core inputs differ.  Each core
emits a partial output (its 4 heads' contribution through Wo); the host adds
the two partials per batch plus bo.

Device-side structure per core:
  - host pre-transposes q/k/v to [E, S] so projections contract over e on
    the partition dim; Wq and bq are pre-scaled by 1/sqrt(dk).
  - qh^T, kh^T stored [128(4h x 32d), S]; vh stored [S, 4h, 33] bf16 with a
    ones column so the AV matmul also yields the softmax-#2 denominator.
  - causal q-block loop (128 queries, extent = (k+1) key-blocks); the
    diagonal block is masked by accumulating ident^T @ triu(-1e30) onto the
    QK PSUM scores.
  - softmax #1 skips the max-subtraction (scores are O(5), fp32 exp is safe);
    the key-axis cumsum is a REVERSED tensor_tensor_scan giving the exact
    suffix-sum (no 1-x cancellation); its col 0 is the softmax denominator.
  - te = exp(-sqrt(gamma^2 * tail * pos / sigma)): gamma^2/sigma ride the
    scalar slot of one scalar_tensor_tensor; sqrt+exp fused across 4 heads.
  - softmax #2: e2 = exp(s * clip(te)) directly (no max, masked lanes are
    exp(-1e30)=0); Sigma2 via the ones column; normalization folded into a
    per-partition tensor_scalar on the AV output.
  - e2 (bf16) transposed for AV by the DMA xbar (sync engine), not PE.
"""

import os
import sys

for _p in ("/opt/trn_rl_repo", "/root/.axon_site/_ro/trn_rl_repo"):
    if os.path.isdir(_p) and _p not in sys.path:
        sys.path.insert(0, _p)

import numpy as np

import concourse.bacc as bacc
import concourse.bass as bass
import concourse.mybir as mybir
from concourse.tile import TileContext

B, S, E, H = 4, 2048, 256, 8
DK = E // H          # 32
HG = 4               # heads per core
D = HG * DK          # 128, per-core projected width
NCORES = 8

FP = mybir.dt.float32
BF = mybir.dt.bfloat16
AF = mybir.ActivationFunctionType
OP = mybir.AluOpType
NEG = -1e30


class _AktBacc(bacc.Bacc):
    """Bacc whose activation-table placement only considers the one set
    covering every ACT function this kernel uses (Exp, Ln, Identity, Copy).
    The default first-match policy alternates exp_and_others with a
    Ln-capable set, reloading the 2.7us ACT tables per tile."""

    _ACT_SET = "natural_log_exp_and_others"

    def insert_act_table_loads(self):
        import concourse.mybir as _mb
        from concourse.hw_specs import get_activation_tables
        has_activation = any(
            isinstance(i, _mb.InstActivation)
            for b in self.main_func.blocks
            for i in b.instructions
        )
        if not has_activation:
            return
        # positions must stay canonical (act_func_set_id indexes this list)
        tables = [
            (nm, fs if nm == self._ACT_SET else set())
            for nm, fs in get_activation_tables(self.m.arch).items()
        ]
        import bass_rust as _br
        _br.insert_act_table_loads(self, tables)


def build_nc(s_len=S, qk_f32r=False):
    """Build the single-core SPMD graph.  s_len parametrizes the sequence
    length for small-scale simulation tests (must be a multiple of 128)."""
    nqb = s_len // 128           # number of 128-query blocks
    nech = E // 128              # e-chunks (2)

    nc = _AktBacc()
    qT = nc.declare_dram_parameter("qT", [E, s_len], FP, isOutput=False)
    kT = nc.declare_dram_parameter("kT", [E, s_len], FP, isOutput=False)
    vT = nc.declare_dram_parameter("vT", [E, s_len], FP, isOutput=False)
    wqT = nc.declare_dram_parameter("wqT", [E, D], FP, isOutput=False)
    wkT = nc.declare_dram_parameter("wkT", [E, D], FP, isOutput=False)
    wvT = nc.declare_dram_parameter("wvT", [E, D], FP, isOutput=False)
    woT = nc.declare_dram_parameter("woT", [D, E], FP, isOutput=False)
    bqs = nc.declare_dram_parameter("bqs", [64, 2], FP, isOutput=False)
    bks = nc.declare_dram_parameter("bks", [64, 2], FP, isOutput=False)
    bvrow = nc.declare_dram_parameter("bvrow", [1, D], FP, isOutput=False)
    gsq = nc.declare_dram_parameter("gsq", [128, HG], FP, isOutput=False)
    out_part = nc.declare_dram_parameter("out_part", [s_len, E], FP, isOutput=True)

    qk_dt = mybir.dt.float32r if qk_f32r else FP

    with TileContext(nc) as tc:
        with (
            tc.tile_pool(name="consts", bufs=1) as consts,
            tc.tile_pool(name="persist", bufs=1) as persist,
        ):
            # ---- constants ----
            ident_f = consts.tile([128, 128], FP)
            nc.vector.memset(ident_f[:], 1.0)
            nc.gpsimd.affine_select(out=ident_f[:], in_=ident_f[:],
                                    compare_op=OP.is_equal, fill=0.0,
                                    base=0, pattern=[[-1, 128]], channel_multiplier=1)
            ident_b = consts.tile([128, 128], BF)
            nc.vector.tensor_copy(out=ident_b[:], in_=ident_f[:])
            # strict upper triangle = NEG, else 0 (diagonal-block causal mask)
            triu_neg = consts.tile([128, 128], BF)
            nc.gpsimd.memset(triu_neg[:], 0.0)
            nc.gpsimd.affine_select(out=triu_neg[:], in_=triu_neg[:],
                                    compare_op=OP.is_ge, fill=NEG,
                                    base=0, pattern=[[-1, 128]], channel_multiplier=1)
            ones1 = consts.tile([1, 128], FP)
            nc.vector.memset(ones1[:], 1.0)

            gsq_sb = consts.tile([128, HG], FP)
            nc.sync.dma_start(out=gsq_sb[:], in_=gsq[:])
            bq_sb = consts.tile([64, 2], FP)
            nc.sync.dma_start(out=bq_sb[:], in_=bqs[:])
            bk_sb = consts.tile([64, 2], FP)
            nc.sync.dma_start(out=bk_sb[:], in_=bks[:])
            bv_sb = consts.tile([1, D], FP)
            nc.sync.dma_start(out=bv_sb[:], in_=bvrow[:])
            wo_sb = consts.tile([D, E], FP)
            nc.sync.dma_start(out=wo_sb[:], in_=woT[:])

            # ---- persistent activations ----
            # head h lives at partitions (h%2)*32..+32, free-block h//2
            # (PE operands may only start at partition 0/32/64)
            qhT = persist.tile([64, 2, s_len], BF)
            khT = persist.tile([64, 2, s_len], BF)
            vh1 = persist.tile([128, nqb, HG, 33], BF)  # [s-part, s-blk, h, 32d+1]
            nc.vector.memset(vh1[:, :, :, 32:33], 1.0)

            # ---- phase 0: projections ----
            with (
                tc.tile_pool(name="ph0", bufs=2) as ph0,
                tc.tile_pool(name="ph0w", bufs=1) as ph0w,
                tc.tile_pool(name="ph0ps", bufs=2, space="PSUM") as ph0ps,
            ):
                wq_sb = ph0w.tile([128, nech, D], FP)
                wk_sb = ph0w.tile([128, nech, D], FP)
                wv_sb = ph0w.tile([128, nech, D], FP)
                nc.sync.dma_start(out=wq_sb[:], in_=wqT.rearrange("(c p) d -> p c d", p=128))
                nc.sync.dma_start(out=wk_sb[:], in_=wkT.rearrange("(c p) d -> p c d", p=128))
                nc.sync.dma_start(out=wv_sb[:], in_=wvT.rearrange("(c p) d -> p c d", p=128))

                for name, src, wsb, bias, dst in (
                    ("q", qT, wq_sb, bq_sb, qhT),
                    ("k", kT, wk_sb, bk_sb, khT),
                ):
                    x_sb = ph0.tile([128, nech, s_len], FP, tag="x_in")
                    nc.sync.dma_start(out=x_sb[:],
                                      in_=src.rearrange("(c p) s -> p c s", p=128))
                    for dg in range(2):          # head-pairs (0,1) and (2,3)
                        for sc in range((s_len + 511) // 512):
                            s0, s1 = sc * 512, min((sc + 1) * 512, s_len)
                            ps = ph0ps.tile([64, 512], FP, tag=f"projps_{name}")
                            for c in range(nech):
                                nc.tensor.matmul(ps[:, 0:s1 - s0],
                                                 lhsT=wsb[:, c, dg * 64:(dg + 1) * 64],
                                                 rhs=x_sb[:, c, s0:s1],
                                                 start=(c == 0), stop=(c == nech - 1))
                            nc.scalar.activation(out=dst[:, dg, s0:s1],
                                                 in_=ps[:, 0:s1 - s0], func=AF.Identity,
                                                 bias=bias[:, dg:dg + 1])

                # vh: natural [s, d] orientation + bias row + bf16 cast
                xv_sb = ph0.tile([128, nech, s_len], FP, tag="x_in")
                nc.sync.dma_start(out=xv_sb[:],
                                  in_=vT.rearrange("(c p) s -> p c s", p=128))
                for sb in range(nqb):
                    ps = ph0ps.tile([128, 128], FP, tag="vps")
                    for c in range(nech):
                        nc.tensor.matmul(ps[:], lhsT=xv_sb[:, c, sb * 128:(sb + 1) * 128],
                                         rhs=wv_sb[:, c, :], start=(c == 0), stop=False)
                    nc.tensor.matmul(ps[:], lhsT=ones1[:], rhs=bv_sb[:],
                                     start=False, stop=True)
                    for h in range(HG):
                        nc.scalar.activation(out=vh1[:, sb, h, 0:32],
                                             in_=ps[:, h * 32:(h + 1) * 32], func=AF.Copy)

            # ---- attention loop ----
            with (
                tc.tile_pool(name="att", bufs=1) as att,
                tc.tile_pool(name="att2", bufs=2) as att2,
                tc.tile_pool(name="att3", bufs=3) as att3,
                tc.tile_pool(name="ps_sm", bufs=1, space="PSUM") as ps_sm,
                tc.tile_pool(name="ps_lg", bufs=1, space="PSUM") as ps_lg,
                tc.tile_pool(name="ps_av", bufs=1, space="PSUM") as ps_av,
                tc.tile_pool(name="ps_op", bufs=1, space="PSUM") as ps_op,
            ):
                # pair small and large extents so the two PSUM score buffers
                # (2 + 4 banks) let two tiles pipeline at once
                half = (nqb + 1) // 2
                pairs = [(lo, nqb - 1 - lo) for lo in range(half)]

                def attn_tile(kq, h, posk, concat):
                    nb = kq + 1            # causal extent in 128-key blocks
                    N = nb * 128
                    small = kq < half
                    pool = ps_sm if small else ps_lg
                    s_ps = pool.tile([128, half * 128 if small else nqb * 128],
                                     FP, tag="s_sm" if small else "s_lg")
                    # QK^T: chunks of <=512 moving columns
                    hp, hb = (h % 2) * 32, h // 2
                    nchunk = (N + 511) // 512
                    for c in range(nchunk):
                        c0, c1 = c * 512, min((c + 1) * 512, N)
                        nc.tensor.matmul(
                            s_ps[:, c0:c1],
                            lhsT=qhT[hp:hp + 32, hb, kq * 128:(kq + 1) * 128],
                            rhs=khT[hp:hp + 32, hb, c0:c1],
                            start=True, stop=True, skip_group_check=True)
                    # diagonal-block causal mask: += I^T @ triu(NEG)
                    nc.tensor.matmul(s_ps[:, N - 128:N], lhsT=ident_b[:],
                                     rhs=triu_neg[:], start=False, stop=True,
                                     skip_group_check=True)

                    # softmax #1 numerator (no max shift needed, |s| is small)
                    e = att3.tile([128, s_len], BF, tag="e")
                    nc.scalar.activation(out=e[:, :N], in_=s_ps[:, :N], func=AF.Exp)

                    # suffix sums: tail[j] = sum_{j'>=j} e[j'] (reversed scan);
                    # tail[0] is the softmax denominator, tail[j+1] the
                    # exclusive tail the decay term needs
                    tail = att3.tile([128, s_len + 2], BF, tag="tail")
                    nc.gpsimd.memset(tail[:, N:N + 1], 0.0)
                    nc.vector.tensor_tensor_scan(
                        out=tail[:, 0:N][:, ::-1], data0=e[:, 0:N][:, ::-1],
                        data1=e[:, 0:N][:, ::-1], initial=0.0,
                        op0=OP.add, op1=OP.bypass)

                    # tsq = gamma^2 * tail_excl * pos  (sigma bound later)
                    tsq = att2.tile([128, s_len], FP, tag="tsq")
                    nc.vector.scalar_tensor_tensor(
                        out=tsq[:, :N], in0=tail[:, 1:N + 1],
                        scalar=gsq_sb[:, h:h + 1],
                        in1=posk[:, :N], op0=OP.mult, op1=OP.mult)
                    # mrs = -rsqrt(sigma1) = -exp(-0.5*ln(sigma1))   [128,1]
                    mrs = att3.tile([128, 1], FP, tag="mrs")
                    nc.scalar.activation(out=mrs[:], in_=tail[:, 0:1], func=AF.Ln)
                    nc.scalar.activation(out=mrs[:], in_=mrs[:], func=AF.Exp,
                                         scale=-0.5)
                    nc.vector.tensor_scalar(out=mrs[:], in0=mrs[:], scalar1=-1.0,
                                            scalar2=None, op0=OP.mult)
                    # dist = sqrt(tsq) = exp(0.5*ln(tsq)) -- Ln+Exp share one
                    # ACT table set (Sqrt doesn't fit beside Exp);
                    # tsq=+0 -> ln=-inf -> dist=0 -> te=1 exactly
                    nc.scalar.activation(out=tsq[:, :N], in_=tsq[:, :N], func=AF.Ln)
                    nc.scalar.activation(out=tsq[:, :N], in_=tsq[:, :N],
                                         func=AF.Exp, scale=0.5)
                    nc.scalar.activation(out=tsq[:, :N], in_=tsq[:, :N],
                                         func=AF.Exp, scale=mrs[:])

                    # s2 = max(te, 1e-5) * s   (masked lanes stay ~ -1e30)
                    s2 = att2.tile([128, s_len], FP, tag="s2")
                    nc.vector.scalar_tensor_tensor(
                        out=s2[:, :N], in0=tsq[:, :N], scalar=1e-5,
                        in1=s_ps[:, :N], op0=OP.max, op1=OP.mult)
                    # softmax #2 numerator
                    e2 = att2.tile([128, s_len], BF, tag="e2")
                    nc.scalar.activation(out=e2[:, :N], in_=s2[:, :N], func=AF.Exp)

                    e2t = att2.tile([128, nqb, 128], BF, tag="e2t")
                    nc.sync.dma_start_transpose(out=e2t[:, 0:nb, :], in_=e2[:, :N])
                    av = ps_av.tile([128, 64], FP, tag="av")
                    for c in range(nb):
                        nc.tensor.matmul(av[:, 0:33], lhsT=e2t[:, c, :],
                                         rhs=vh1[:, c, h, :],
                                         start=(c == 0), stop=(c == nb - 1))
                    rec2 = att3.tile([128, 1], FP, tag="rec2")
                    nc.vector.reciprocal(out=rec2[:], in_=av[:, 32:33])
                    nc.vector.tensor_scalar(
                        out=concat[:, h * 32:(h + 1) * 32], in0=av[:, 0:32],
                        scalar1=rec2[:], scalar2=None, op0=OP.mult)

                def out_proj(kq, concat):
                    trp = ps_op.tile([128, 128], FP, tag="trop")
                    nc.tensor.transpose(out=trp[:], in_=concat[:], identity=ident_f[:])
                    concatT = att2.tile([128, 128], FP, tag="concatT")
                    nc.scalar.activation(out=concatT[:], in_=trp[:], func=AF.Copy)
                    op = ps_op.tile([128, 256], FP, tag="trop")
                    nc.tensor.matmul(op[:], lhsT=concatT[:], rhs=wo_sb[:],
                                     start=True, stop=True)
                    ostg = att2.tile([128, 256], FP, tag="ostg")
                    nc.scalar.activation(out=ostg[:], in_=op[:], func=AF.Copy)
                    nc.sync.dma_start(out=out_part[kq * 128:(kq + 1) * 128, :],
                                      in_=ostg[:])

                for ksm, klg in pairs:
                    posk_s = att2.tile([128, half * 128], FP, tag="pos_s")
                    nc.gpsimd.iota(posk_s[:, :(ksm + 1) * 128],
                                   pattern=[[-1, (ksm + 1) * 128]], base=128 * ksm,
                                   channel_multiplier=1,
                                   allow_small_or_imprecise_dtypes=True)
                    nc.gpsimd.affine_select(
                        out=posk_s[:, ksm * 128:(ksm + 1) * 128],
                        in_=posk_s[:, ksm * 128:(ksm + 1) * 128],
                        compare_op=OP.is_ge, fill=0.0, base=0,
                        pattern=[[-1, 128]], channel_multiplier=1)
                    concat_s = att2.tile([128, 128], FP, tag="concat_s")
                    if klg != ksm:
                        posk_l = att2.tile([128, nqb * 128], FP, tag="pos_l")
                        nc.gpsimd.iota(posk_l[:, :(klg + 1) * 128],
                                       pattern=[[-1, (klg + 1) * 128]], base=128 * klg,
                                       channel_multiplier=1,
                                       allow_small_or_imprecise_dtypes=True)
                        nc.gpsimd.affine_select(
                            out=posk_l[:, klg * 128:(klg + 1) * 128],
                            in_=posk_l[:, klg * 128:(klg + 1) * 128],
                            compare_op=OP.is_ge, fill=0.0, base=0,
                            pattern=[[-1, 128]], channel_multiplier=1)
                        concat_l = att2.tile([128, 128], FP, tag="concat_l")
                    for h in range(HG):
                        attn_tile(ksm, h, posk_s, concat_s)
                        if klg != ksm:
                            attn_tile(klg, h, posk_l, concat_l)
                    out_proj(ksm, concat_s)
                    if klg != ksm:
                        out_proj(klg, concat_l)
    return nc


# ---------------------------------------------------------------------------
# host side
# ---------------------------------------------------------------------------

def _softplus(x):
    return np.logaddexp(0.0, x)


def _make_in_maps(q, k, v, Wq, bq, Wk, bk, Wv, bv, Wo, gammas, s_len=S):
    scale = 1.0 / np.sqrt(np.float32(DK))
    g = -_softplus(gammas.reshape(H).astype(np.float64)).astype(np.float32)
    in_maps = []
    for core in range(NCORES):
        b, grp = core // 2, core % 2
        hsel = slice(grp * HG * DK, (grp + 1) * HG * DK)   # rows of W, dims of proj
        gam = g[grp * HG:(grp + 1) * HG]
        in_maps.append({
            "qT": np.ascontiguousarray(q[b].T.astype(np.float32)),
            "kT": np.ascontiguousarray(k[b].T.astype(np.float32)),
            "vT": np.ascontiguousarray(v[b].T.astype(np.float32)),
            "wqT": np.ascontiguousarray((Wq[hsel, :] * scale).T.astype(np.float32)),
            "wkT": np.ascontiguousarray(Wk[hsel, :].T.astype(np.float32)),
            "wvT": np.ascontiguousarray(Wv[hsel, :].T.astype(np.float32)),
            "woT": np.ascontiguousarray(Wo[:, hsel].T.astype(np.float32)),
            "bqs": np.ascontiguousarray(
                (bq[hsel] * scale).astype(np.float32).reshape(2, 64).T),
            "bks": np.ascontiguousarray(
                bk[hsel].astype(np.float32).reshape(2, 64).T),
            "bvrow": bv[hsel].astype(np.float32).reshape(1, D),
            "gsq": np.broadcast_to((gam * gam).astype(np.float32), (128, HG)).copy(),
        })
    return in_maps


_NC_CACHE = {}


def _get_nc(s_len=S):
    if s_len not in _NC_CACHE:
        nc = build_nc(s_len)
        nc.finalize()      # Bacc pipeline: wait splitting, reg alloc, DCE
        _NC_CACHE[s_len] = nc
    return _NC_CACHE[s_len]


def kernel(q, k, v, mask, Wq, bq, Wk, bk, Wv, bv, Wo, bo, gammas):
    """Full-input, full-output entry point.  `mask` is the causal mask the
    reference builds; the kernel hardcodes causality."""
    from concourse.bass_utils import run_bass_kernel_spmd

    q, k, v = (np.asarray(a, np.float32) for a in (q, k, v))
    in_maps = _make_in_maps(q, k, v, np.asarray(Wq), np.asarray(bq),
                            np.asarray(Wk), np.asarray(bk), np.asarray(Wv),
                            np.asarray(bv), np.asarray(Wo),
                            np.asarray(gammas))
    nc = _get_nc(S)
    res = run_bass_kernel_spmd(nc, in_maps, core_ids=list(range(NCORES)))
    parts = [res.results[c]["out_part"] for c in range(NCORES)]
    out = np.empty((B, S, E), np.float32)
    bo = np.asarray(bo, np.float32)
    for b in range(B):
        out[b] = parts[2 * b] + parts[2 * b + 1] + bo[None, :]
    return out


# revision 23
# speedup vs baseline: 1.4430x; 1.2441x over previous
"""Trainium2 Bass kernel for the AKT (attention-with-distance-decay) problem.

Reference math (per batch b, head h, dk=32, S=2048, E=256):
    qh, kh, vh = per-head projections of q,k,v
    s  = qh @ kh^T / sqrt(dk)                    (causal-masked)
    p  = softmax(s)                              (softmax #1)
    tail[j] = sum_{j'>j} p[j']                   (1 - cumsum)
    dist = sqrt(clip(tail * (i-j), 0))
    te   = clip(exp(-softplus(gamma_h) * dist), 1e-5, 1e5)
    attn = softmax(where(mask, s*te, -inf))      (softmax #2)
    out  = (attn @ vh)  -> concat heads -> @ Wo^T + bo

Sharding: 8 cores = (batch b = core//2) x (head-group g = core%2, 4 heads each).
Every core runs the identical graph (SPMD); per-core inputs differ.  Each core
emits a partial output (its 4 heads' contribution through Wo); the host adds
the two partials per batch plus bo.

Device-side structure per core:
  - host pre-transposes q/k/v to [E, S] so projections contract over e on
    the partition dim; Wq and bq are pre-scaled by 1/sqrt(dk).
  - qh^T, kh^T stored [128(4h x 32d), S]; vh stored [S, 4h, 33] bf16 with a
    ones column so the AV matmul also yields the softmax-#2 denominator.
  - causal q-block loop (128 queries, extent = (k+1) key-blocks); the
    diagonal block is masked by accumulating ident^T @ triu(-1e30) onto the
    QK PSUM scores.
  - softmax #1 skips the max-subtraction (scores are O(5), fp32 exp is safe);
    the key-axis cumsum is a REVERSED tensor_tensor_scan giving the exact
    suffix-sum (no 1-x cancellation); its col 0 is the softmax denominator.
  - te = exp(-sqrt(gamma^2 * tail * pos / sigma)): gamma^2/sigma ride the
    scalar slot of one scalar_tensor_tensor; sqrt+exp fused across 4 heads.
  - softmax #2: e2 = exp(s * clip(te)) directly (no max, masked lanes are
    exp(-1e30)=0); Sigma2 via the ones column; normalization folded into a
    per-partition tensor_scalar on the AV output.
  - e2 (bf16) transposed for AV by the DMA xbar (sync engine), not PE.
"""

import os
import sys

for _p in ("/opt/trn_rl_repo", "/root/.axon_site/_ro/trn_rl_repo"):
    if os.path.isdir(_p) and _p not in sys.path:
        sys.path.insert(0, _p)

import numpy as np

import concourse.bacc as bacc
import concourse.bass as bass
import concourse.mybir as mybir
from concourse.tile import TileContext

B, S, E, H = 4, 2048, 256, 8
DK = E // H          # 32
HG = 4               # heads per core
D = HG * DK          # 128, per-core projected width
NCORES = 8

FP = mybir.dt.float32
BF = mybir.dt.bfloat16
AF = mybir.ActivationFunctionType
OP = mybir.AluOpType
NEG = -1e30


class _AktBacc(bacc.Bacc):
    """Bacc whose activation-table placement only considers the one set
    covering every ACT function this kernel uses (Exp, Ln, Identity, Copy).
    The default first-match policy alternates exp_and_others with a
    Ln-capable set, reloading the 2.7us ACT tables per tile."""

    _ACT_SET = "natural_log_exp_and_others"

    def insert_act_table_loads(self):
        import concourse.mybir as _mb
        from concourse.hw_specs import get_activation_tables
        has_activation = any(
            isinstance(i, _mb.InstActivation)
            for b in self.main_func.blocks
            for i in b.instructions
        )
        if not has_activation:
            return
        # positions must stay canonical (act_func_set_id indexes this list)
        tables = [
            (nm, fs if nm == self._ACT_SET else set())
            for nm, fs in get_activation_tables(self.m.arch).items()
        ]
        import bass_rust as _br
        _br.insert_act_table_loads(self, tables)


def build_nc(s_len=S, qk_f32r=False):
    """Build the single-core SPMD graph.  s_len parametrizes the sequence
    length for small-scale simulation tests (must be a multiple of 128)."""
    nqb = s_len // 128           # number of 128-query blocks
    nech = E // 128              # e-chunks (2)

    nc = _AktBacc()
    qT = nc.declare_dram_parameter("qT", [E, s_len], FP, isOutput=False)
    kT = nc.declare_dram_parameter("kT", [E, s_len], FP, isOutput=False)
    vT = nc.declare_dram_parameter("vT", [E, s_len], FP, isOutput=False)
    wqT = nc.declare_dram_parameter("wqT", [E, D], FP, isOutput=False)
    wkT = nc.declare_dram_parameter("wkT", [E, D], FP, isOutput=False)
    wvT = nc.declare_dram_parameter("wvT", [E, D], FP, isOutput=False)
    woT = nc.declare_dram_parameter("woT", [D, E], FP, isOutput=False)
    bqs = nc.declare_dram_parameter("bqs", [64, 2], FP, isOutput=False)
    bks = nc.declare_dram_parameter("bks", [64, 2], FP, isOutput=False)
    bvrow = nc.declare_dram_parameter("bvrow", [1, D], FP, isOutput=False)
    lngsq = nc.declare_dram_parameter("lngsq", [128, HG], FP, isOutput=False)
    out_part = nc.declare_dram_parameter("out_part", [s_len, E], FP, isOutput=True)

    qk_dt = mybir.dt.float32r if qk_f32r else FP

    with TileContext(nc) as tc:
        with (
            tc.tile_pool(name="consts", bufs=1) as consts,
            tc.tile_pool(name="persist", bufs=1) as persist,
        ):
            # ---- constants ----
            ident_f = consts.tile([128, 128], FP)
            nc.vector.memset(ident_f[:], 1.0)
            nc.gpsimd.affine_select(out=ident_f[:], in_=ident_f[:],
                                    compare_op=OP.is_equal, fill=0.0,
                                    base=0, pattern=[[-1, 128]], channel_multiplier=1)
            ident_b = consts.tile([128, 128], BF)
            nc.vector.tensor_copy(out=ident_b[:], in_=ident_f[:])
            # strict upper triangle = NEG, else 0 (diagonal-block causal mask)
            triu_neg = consts.tile([128, 128], BF)
            nc.gpsimd.memset(triu_neg[:], 0.0)
            nc.gpsimd.affine_select(out=triu_neg[:], in_=triu_neg[:],
                                    compare_op=OP.is_ge, fill=NEG,
                                    base=0, pattern=[[-1, 128]], channel_multiplier=1)
            ones1 = consts.tile([1, 128], FP)
            nc.vector.memset(ones1[:], 1.0)

            lngsq_sb = consts.tile([128, HG], FP)
            nc.sync.dma_start(out=lngsq_sb[:], in_=lngsq[:])
            bq_sb = consts.tile([64, 2], FP)
            nc.sync.dma_start(out=bq_sb[:], in_=bqs[:])
            bk_sb = consts.tile([64, 2], FP)
            nc.sync.dma_start(out=bk_sb[:], in_=bks[:])
            bv_sb = consts.tile([1, D], FP)
            nc.sync.dma_start(out=bv_sb[:], in_=bvrow[:])
            wo_sb = consts.tile([D, E], FP)
            nc.sync.dma_start(out=wo_sb[:], in_=woT[:])

            # master ln(pos) table: lnpos_k[:, j] = M[:, 127 + 128k - j]
            # (a reversed AP view), M[r, c] = ln(r + c - 127), -inf at pos<=0
            lnposM = persist.tile([128, s_len], FP)
            nc.gpsimd.iota(lnposM[:], pattern=[[1, s_len]], base=-127,
                           channel_multiplier=1,
                           allow_small_or_imprecise_dtypes=True)
            nc.gpsimd.affine_select(out=lnposM[:], in_=lnposM[:],
                                    compare_op=OP.is_ge, fill=0.0,
                                    base=-127, pattern=[[1, s_len]],
                                    channel_multiplier=1)
            nc.scalar.activation(out=lnposM[:], in_=lnposM[:], func=AF.Ln)

            # ---- persistent activations ----
            # head h lives at partitions (h%2)*32..+32, free-block h//2
            # (PE operands may only start at partition 0/32/64)
            qhT = persist.tile([64, 2, s_len], BF)
            khT = persist.tile([64, 2, s_len], BF)
            vh1 = persist.tile([128, nqb, HG, 33], BF)  # [s-part, s-blk, h, 32d+1]
            nc.vector.memset(vh1[:, :, :, 32:33], 1.0)

            # ---- phase 0: projections ----
            with (
                tc.tile_pool(name="ph0", bufs=2) as ph0,
                tc.tile_pool(name="ph0w", bufs=1) as ph0w,
                tc.tile_pool(name="ph0ps", bufs=2, space="PSUM") as ph0ps,
            ):
                wq_sb = ph0w.tile([128, nech, D], FP)
                wk_sb = ph0w.tile([128, nech, D], FP)
                wv_sb = ph0w.tile([128, nech, D], FP)
                nc.sync.dma_start(out=wq_sb[:], in_=wqT.rearrange("(c p) d -> p c d", p=128))
                nc.sync.dma_start(out=wk_sb[:], in_=wkT.rearrange("(c p) d -> p c d", p=128))
                nc.sync.dma_start(out=wv_sb[:], in_=wvT.rearrange("(c p) d -> p c d", p=128))

                for name, src, wsb, bias, dst in (
                    ("q", qT, wq_sb, bq_sb, qhT),
                    ("k", kT, wk_sb, bk_sb, khT),
                ):
                    x_sb = ph0.tile([128, nech, s_len], FP, tag="x_in")
                    nc.sync.dma_start(out=x_sb[:],
                                      in_=src.rearrange("(c p) s -> p c s", p=128))
                    for dg in range(2):          # head-pairs (0,1) and (2,3)
                        for sc in range((s_len + 511) // 512):
                            s0, s1 = sc * 512, min((sc + 1) * 512, s_len)
                            ps = ph0ps.tile([64, 512], FP, tag=f"projps_{name}")
                            for c in range(nech):
                                nc.tensor.matmul(ps[:, 0:s1 - s0],
                                                 lhsT=wsb[:, c, dg * 64:(dg + 1) * 64],
                                                 rhs=x_sb[:, c, s0:s1],
                                                 start=(c == 0), stop=(c == nech - 1))
                            nc.scalar.activation(out=dst[:, dg, s0:s1],
                                                 in_=ps[:, 0:s1 - s0], func=AF.Identity,
                                                 bias=bias[:, dg:dg + 1])

                # vh: natural [s, d] orientation + bias row + bf16 cast
                xv_sb = ph0.tile([128, nech, s_len], FP, tag="x_in")
                nc.sync.dma_start(out=xv_sb[:],
                                  in_=vT.rearrange("(c p) s -> p c s", p=128))
                for sb in range(nqb):
                    ps = ph0ps.tile([128, 128], FP, tag="vps")
                    for c in range(nech):
                        nc.tensor.matmul(ps[:], lhsT=xv_sb[:, c, sb * 128:(sb + 1) * 128],
                                         rhs=wv_sb[:, c, :], start=(c == 0), stop=False)
                    nc.tensor.matmul(ps[:], lhsT=ones1[:], rhs=bv_sb[:],
                                     start=False, stop=True)
                    for h in range(HG):
                        nc.scalar.activation(out=vh1[:, sb, h, 0:32],
                                             in_=ps[:, h * 32:(h + 1) * 32], func=AF.Copy)

            # ---- attention loop ----
            with (
                tc.tile_pool(name="att", bufs=1) as att,
                tc.tile_pool(name="att2", bufs=2) as att2,
                tc.tile_pool(name="att3", bufs=3) as att3,
                tc.tile_pool(name="ps_sm", bufs=1, space="PSUM") as ps_sm,
                tc.tile_pool(name="ps_lg", bufs=1, space="PSUM") as ps_lg,
                tc.tile_pool(name="ps_av", bufs=1, space="PSUM") as ps_av,
                tc.tile_pool(name="ps_op", bufs=1, space="PSUM") as ps_op,
            ):
                # pair small and large extents so the two PSUM score buffers
                # (2 + 4 banks) let two tiles pipeline at once
                half = (nqb + 1) // 2
                pairs = [(lo, nqb - 1 - lo) for lo in range(half)]

                def attn_tile(kq, h, concat):
                    nb = kq + 1            # causal extent in 128-key blocks
                    N = nb * 128
                    small = kq < half
                    pool = ps_sm if small else ps_lg
                    s_ps = pool.tile([128, half * 128 if small else nqb * 128],
                                     FP, tag="s_sm" if small else "s_lg")
                    # QK^T: chunks of <=512 moving columns
                    hp, hb = (h % 2) * 32, h // 2
                    nchunk = (N + 511) // 512
                    for c in range(nchunk):
                        c0, c1 = c * 512, min((c + 1) * 512, N)
                        nc.tensor.matmul(
                            s_ps[:, c0:c1],
                            lhsT=qhT[hp:hp + 32, hb, kq * 128:(kq + 1) * 128],
                            rhs=khT[hp:hp + 32, hb, c0:c1],
                            start=True, stop=True, skip_group_check=True)
                    # diagonal-block causal mask: += I^T @ triu(NEG)
                    nc.tensor.matmul(s_ps[:, N - 128:N], lhsT=ident_b[:],
                                     rhs=triu_neg[:], start=False, stop=True,
                                     skip_group_check=True)

                    # softmax #1 numerator (no max shift needed, |s| is small)
                    e = att3.tile([128, s_len], BF, tag="e")
                    nc.scalar.activation(out=e[:, :N], in_=s_ps[:, :N], func=AF.Exp)

                    # suffix sums: tail[j] = sum_{j'>=j} e[j'] (reversed scan);
                    # tail[0] is the softmax denominator, tail[j+1] the
                    # exclusive tail the decay term needs
                    tail = att3.tile([128, s_len + 2], BF, tag="tail")
                    nc.gpsimd.memset(tail[:, N:N + 1], 0.0)
                    nc.vector.tensor_tensor_scan(
                        out=tail[:, 0:N][:, ::-1], data0=e[:, 0:N][:, ::-1],
                        data1=e[:, 0:N][:, ::-1], initial=0.0,
                        op0=OP.add, op1=OP.bypass)

                    # log-domain decay: dist = exp(0.5*(ln tail + ln pos
                    # + ln gamma^2 - ln sigma)); Ln+Exp share one ACT table
                    # set (Sqrt doesn't fit beside Exp); tail or pos = +0
                    # gives -inf -> dist=0 -> te=1 exactly
                    lnt = att2.tile([128, s_len + 2], FP, tag="lnt")
                    nc.scalar.activation(out=lnt[:, 0:N + 1], in_=tail[:, 0:N + 1],
                                         func=AF.Ln)
                    # c_h = ln gamma_h^2 - ln sigma1   [128,1]
                    ch = att3.tile([128, 1], FP, tag="ch")
                    nc.vector.tensor_scalar(out=ch[:], in0=lnt[:, 0:1],
                                            scalar1=-1.0,
                                            scalar2=lngsq_sb[:, h:h + 1],
                                            op0=OP.mult, op1=OP.add)
                    # u = ln(tail_excl) + c_h + ln(pos), in place (cols 1..N)
                    nc.vector.scalar_tensor_tensor(
                        out=lnt[:, 1:N + 1], in0=lnt[:, 1:N + 1], scalar=ch[:],
                        in1=lnposM[:, 127 + 128 * kq::-1],
                        op0=OP.add, op1=OP.add)
                    nc.scalar.activation(out=lnt[:, 1:N + 1], in_=lnt[:, 1:N + 1],
                                         func=AF.Exp, scale=0.5)
                    nc.scalar.activation(out=lnt[:, 1:N + 1], in_=lnt[:, 1:N + 1],
                                         func=AF.Exp, scale=-1.0)

                    # s2 = max(te, 1e-5) * s   (masked lanes stay ~ -1e30)
                    s2 = att2.tile([128, s_len], FP, tag="s2")
                    nc.vector.scalar_tensor_tensor(
                        out=s2[:, :N], in0=lnt[:, 1:N + 1], scalar=1e-5,
                        in1=s_ps[:, :N], op0=OP.max, op1=OP.mult)
                    # softmax #2 numerator
                    e2 = att2.tile([128, s_len], BF, tag="e2")
                    nc.scalar.activation(out=e2[:, :N], in_=s2[:, :N], func=AF.Exp)

                    e2t = att2.tile([128, nqb, 128], BF, tag="e2t")
                    nc.sync.dma_start_transpose(out=e2t[:, 0:nb, :], in_=e2[:, :N])
                    av = ps_av.tile([128, 64], FP, tag="av")
                    for c in range(nb):
                        nc.tensor.matmul(av[:, 0:33], lhsT=e2t[:, c, :],
                                         rhs=vh1[:, c, h, :],
                                         start=(c == 0), stop=(c == nb - 1))
                    rec2 = att3.tile([128, 1], FP, tag="rec2")
                    nc.vector.reciprocal(out=rec2[:], in_=av[:, 32:33])
                    nc.vector.tensor_scalar(
                        out=concat[:, h * 32:(h + 1) * 32], in0=av[:, 0:32],
                        scalar1=rec2[:], scalar2=None, op0=OP.mult)

                def out_proj(kq, concat):
                    trp = ps_op.tile([128, 128], FP, tag="trop")
                    nc.tensor.transpose(out=trp[:], in_=concat[:], identity=ident_f[:])
                    concatT = att2.tile([128, 128], FP, tag="concatT")
                    nc.scalar.activation(out=concatT[:], in_=trp[:], func=AF.Copy)
                    op = ps_op.tile([128, 256], FP, tag="trop")
                    nc.tensor.matmul(op[:], lhsT=concatT[:], rhs=wo_sb[:],
                                     start=True, stop=True)
                    ostg = att2.tile([128, 256], FP, tag="ostg")
                    nc.scalar.activation(out=ostg[:], in_=op[:], func=AF.Copy)
                    nc.sync.dma_start(out=out_part[kq * 128:(kq + 1) * 128, :],
                                      in_=ostg[:])

                for ksm, klg in pairs:
                    concat_s = att2.tile([128, 128], FP, tag="concat_s")
                    if klg != ksm:
                        concat_l = att2.tile([128, 128], FP, tag="concat_l")
                    for h in range(HG):
                        attn_tile(ksm, h, concat_s)
                        if klg != ksm:
                            attn_tile(klg, h, concat_l)
                    out_proj(ksm, concat_s)
                    if klg != ksm:
                        out_proj(klg, concat_l)
    return nc


# ---------------------------------------------------------------------------
# host side
# ---------------------------------------------------------------------------

def _softplus(x):
    return np.logaddexp(0.0, x)


def _make_in_maps(q, k, v, Wq, bq, Wk, bk, Wv, bv, Wo, gammas, s_len=S):
    scale = 1.0 / np.sqrt(np.float32(DK))
    g = -_softplus(gammas.reshape(H).astype(np.float64)).astype(np.float32)
    in_maps = []
    for core in range(NCORES):
        b, grp = core // 2, core % 2
        hsel = slice(grp * HG * DK, (grp + 1) * HG * DK)   # rows of W, dims of proj
        gam = g[grp * HG:(grp + 1) * HG]
        in_maps.append({
            "qT": np.ascontiguousarray(q[b].T.astype(np.float32)),
            "kT": np.ascontiguousarray(k[b].T.astype(np.float32)),
            "vT": np.ascontiguousarray(v[b].T.astype(np.float32)),
            "wqT": np.ascontiguousarray((Wq[hsel, :] * scale).T.astype(np.float32)),
            "wkT": np.ascontiguousarray(Wk[hsel, :].T.astype(np.float32)),
            "wvT": np.ascontiguousarray(Wv[hsel, :].T.astype(np.float32)),
            "woT": np.ascontiguousarray(Wo[:, hsel].T.astype(np.float32)),
            "bqs": np.ascontiguousarray(
                (bq[hsel] * scale).astype(np.float32).reshape(2, 64).T),
            "bks": np.ascontiguousarray(
                bk[hsel].astype(np.float32).reshape(2, 64).T),
            "bvrow": bv[hsel].astype(np.float32).reshape(1, D),
            "lngsq": np.broadcast_to(
                (2.0 * np.log(-gam)).astype(np.float32), (128, HG)).copy(),
        })
    return in_maps


_NC_CACHE = {}


def _get_nc(s_len=S):
    if s_len not in _NC_CACHE:
        nc = build_nc(s_len)
        nc.finalize()      # Bacc pipeline: wait splitting, reg alloc, DCE
        _NC_CACHE[s_len] = nc
    return _NC_CACHE[s_len]


def kernel(q, k, v, mask, Wq, bq, Wk, bk, Wv, bv, Wo, bo, gammas):
    """Full-input, full-output entry point.  `mask` is the causal mask the
    reference builds; the kernel hardcodes causality."""
    from concourse.bass_utils import run_bass_kernel_spmd

    q, k, v = (np.asarray(a, np.float32) for a in (q, k, v))
    in_maps = _make_in_maps(q, k, v, np.asarray(Wq), np.asarray(bq),
                            np.asarray(Wk), np.asarray(bk), np.asarray(Wv),
                            np.asarray(bv), np.asarray(Wo),
                            np.asarray(gammas))
    nc = _get_nc(S)
    res = run_bass_kernel_spmd(nc, in_maps, core_ids=list(range(NCORES)))
    parts = [res.results[c]["out_part"] for c in range(NCORES)]
    out = np.empty((B, S, E), np.float32)
    bo = np.asarray(bo, np.float32)
    for b in range(B):
        out[b] = parts[2 * b] + parts[2 * b + 1] + bo[None, :]
    return out


# revision 24
# speedup vs baseline: 1.4479x; 1.0034x over previous
"""Trainium2 Bass kernel for the AKT (attention-with-distance-decay) problem.

Reference math (per batch b, head h, dk=32, S=2048, E=256):
    qh, kh, vh = per-head projections of q,k,v
    s  = qh @ kh^T / sqrt(dk)                    (causal-masked)
    p  = softmax(s)                              (softmax #1)
    tail[j] = sum_{j'>j} p[j']                   (1 - cumsum)
    dist = sqrt(clip(tail * (i-j), 0))
    te   = clip(exp(-softplus(gamma_h) * dist), 1e-5, 1e5)
    attn = softmax(where(mask, s*te, -inf))      (softmax #2)
    out  = (attn @ vh)  -> concat heads -> @ Wo^T + bo

Sharding: 8 cores = (batch b = core//2) x (head-group g = core%2, 4 heads each).
Every core runs the identical graph (SPMD); per-core inputs differ.  Each core
emits a partial output (its 4 heads' contribution through Wo); the host adds
the two partials per batch plus bo.

Device-side structure per core:
  - host pre-transposes q/k/v to [E, S] so projections contract over e on
    the partition dim; Wq and bq are pre-scaled by 1/sqrt(dk).
  - qh^T, kh^T stored [128(4h x 32d), S]; vh stored [S, 4h, 33] bf16 with a
    ones column so the AV matmul also yields the softmax-#2 denominator.
  - causal q-block loop (128 queries, extent = (k+1) key-blocks); the
    diagonal block is masked by accumulating ident^T @ triu(-1e30) onto the
    QK PSUM scores.
  - softmax #1 skips the max-subtraction (scores are O(5), fp32 exp is safe);
    the key-axis cumsum is a REVERSED tensor_tensor_scan giving the exact
    suffix-sum (no 1-x cancellation); its col 0 is the softmax denominator.
  - te = exp(-sqrt(gamma^2 * tail * pos / sigma)): gamma^2/sigma ride the
    scalar slot of one scalar_tensor_tensor; sqrt+exp fused across 4 heads.
  - softmax #2: e2 = exp(s * clip(te)) directly (no max, masked lanes are
    exp(-1e30)=0); Sigma2 via the ones column; normalization folded into a
    per-partition tensor_scalar on the AV output.
  - e2 (bf16) transposed for AV by the DMA xbar (sync engine), not PE.
"""

import os
import sys

for _p in ("/opt/trn_rl_repo", "/root/.axon_site/_ro/trn_rl_repo"):
    if os.path.isdir(_p) and _p not in sys.path:
        sys.path.insert(0, _p)

import numpy as np

import concourse.bacc as bacc
import concourse.bass as bass
import concourse.mybir as mybir
from concourse.tile import TileContext

B, S, E, H = 4, 2048, 256, 8
DK = E // H          # 32
HG = 4               # heads per core
D = HG * DK          # 128, per-core projected width
NCORES = 8

FP = mybir.dt.float32
BF = mybir.dt.bfloat16
AF = mybir.ActivationFunctionType
OP = mybir.AluOpType
NEG = -1e30


class _AktBacc(bacc.Bacc):
    """Bacc whose activation-table placement only considers the one set
    covering every ACT function this kernel uses (Exp, Ln, Identity, Copy).
    The default first-match policy alternates exp_and_others with a
    Ln-capable set, reloading the 2.7us ACT tables per tile."""

    _ACT_SET = "natural_log_exp_and_others"

    def insert_act_table_loads(self):
        import concourse.mybir as _mb
        from concourse.hw_specs import get_activation_tables
        has_activation = any(
            isinstance(i, _mb.InstActivation)
            for b in self.main_func.blocks
            for i in b.instructions
        )
        if not has_activation:
            return
        # positions must stay canonical (act_func_set_id indexes this list)
        tables = [
            (nm, fs if nm == self._ACT_SET else set())
            for nm, fs in get_activation_tables(self.m.arch).items()
        ]
        import bass_rust as _br
        _br.insert_act_table_loads(self, tables)


def build_nc(s_len=S, qk_f32r=False):
    """Build the single-core SPMD graph.  s_len parametrizes the sequence
    length for small-scale simulation tests (must be a multiple of 128)."""
    nqb = s_len // 128           # number of 128-query blocks
    nech = E // 128              # e-chunks (2)

    nc = _AktBacc()
    qT = nc.declare_dram_parameter("qT", [E, s_len], FP, isOutput=False)
    kT = nc.declare_dram_parameter("kT", [E, s_len], FP, isOutput=False)
    vT = nc.declare_dram_parameter("vT", [E, s_len], FP, isOutput=False)
    wqT = nc.declare_dram_parameter("wqT", [E, D], FP, isOutput=False)
    wkT = nc.declare_dram_parameter("wkT", [E, D], FP, isOutput=False)
    wvT = nc.declare_dram_parameter("wvT", [E, D], FP, isOutput=False)
    woT = nc.declare_dram_parameter("woT", [D, E], FP, isOutput=False)
    bqs = nc.declare_dram_parameter("bqs", [64, 2], FP, isOutput=False)
    bks = nc.declare_dram_parameter("bks", [64, 2], FP, isOutput=False)
    bvrow = nc.declare_dram_parameter("bvrow", [1, D], FP, isOutput=False)
    lngsq = nc.declare_dram_parameter("lngsq", [128, HG], FP, isOutput=False)
    out_part = nc.declare_dram_parameter("out_part", [s_len, E], FP, isOutput=True)

    qk_dt = mybir.dt.float32r if qk_f32r else FP

    with TileContext(nc) as tc:
        with (
            tc.tile_pool(name="consts", bufs=1) as consts,
            tc.tile_pool(name="persist", bufs=1) as persist,
        ):
            # ---- constants ----
            ident_f = consts.tile([128, 128], FP)
            nc.vector.memset(ident_f[:], 1.0)
            nc.gpsimd.affine_select(out=ident_f[:], in_=ident_f[:],
                                    compare_op=OP.is_equal, fill=0.0,
                                    base=0, pattern=[[-1, 128]], channel_multiplier=1)
            ident_b = consts.tile([128, 128], BF)
            nc.vector.tensor_copy(out=ident_b[:], in_=ident_f[:])
            # strict upper triangle = NEG, else 0 (diagonal-block causal mask)
            triu_neg = consts.tile([128, 128], BF)
            nc.gpsimd.memset(triu_neg[:], 0.0)
            nc.gpsimd.affine_select(out=triu_neg[:], in_=triu_neg[:],
                                    compare_op=OP.is_ge, fill=NEG,
                                    base=0, pattern=[[-1, 128]], channel_multiplier=1)
            ones1 = consts.tile([1, 128], FP)
            nc.vector.memset(ones1[:], 1.0)

            lngsq_sb = consts.tile([128, HG], FP)
            nc.sync.dma_start(out=lngsq_sb[:], in_=lngsq[:])
            bq_sb = consts.tile([64, 2], FP)
            nc.sync.dma_start(out=bq_sb[:], in_=bqs[:])
            bk_sb = consts.tile([64, 2], FP)
            nc.sync.dma_start(out=bk_sb[:], in_=bks[:])
            bv_sb = consts.tile([1, D], FP)
            nc.sync.dma_start(out=bv_sb[:], in_=bvrow[:])
            wo_sb = consts.tile([D, E], FP)
            nc.sync.dma_start(out=wo_sb[:], in_=woT[:])

            # master ln(pos) table: lnpos_k[:, j] = M[:, 127 + 128k - j]
            # (a reversed AP view), M[r, c] = ln(r + c - 127), -inf at pos<=0
            lnposM = persist.tile([128, s_len], FP)
            nc.gpsimd.iota(lnposM[:], pattern=[[1, s_len]], base=-127,
                           channel_multiplier=1,
                           allow_small_or_imprecise_dtypes=True)
            nc.gpsimd.affine_select(out=lnposM[:], in_=lnposM[:],
                                    compare_op=OP.is_ge, fill=0.0,
                                    base=-127, pattern=[[1, s_len]],
                                    channel_multiplier=1)
            nc.scalar.activation(out=lnposM[:], in_=lnposM[:], func=AF.Ln)

            # ---- persistent activations ----
            # head h lives at partitions (h%2)*32..+32, free-block h//2
            # (PE operands may only start at partition 0/32/64)
            qhT = persist.tile([64, 2, s_len], BF)
            khT = persist.tile([64, 2, s_len], BF)
            vh1 = persist.tile([128, nqb, HG, 33], BF)  # [s-part, s-blk, h, 32d+1]
            nc.vector.memset(vh1[:, :, :, 32:33], 1.0)

            # ---- phase 0: projections ----
            with (
                tc.tile_pool(name="ph0", bufs=2) as ph0,
                tc.tile_pool(name="ph0w", bufs=1) as ph0w,
                tc.tile_pool(name="ph0ps", bufs=2, space="PSUM") as ph0ps,
            ):
                wq_sb = ph0w.tile([128, nech, D], FP)
                wk_sb = ph0w.tile([128, nech, D], FP)
                wv_sb = ph0w.tile([128, nech, D], FP)
                nc.sync.dma_start(out=wq_sb[:], in_=wqT.rearrange("(c p) d -> p c d", p=128))
                nc.sync.dma_start(out=wk_sb[:], in_=wkT.rearrange("(c p) d -> p c d", p=128))
                nc.sync.dma_start(out=wv_sb[:], in_=wvT.rearrange("(c p) d -> p c d", p=128))

                for name, src, wsb, bias, dst in (
                    ("q", qT, wq_sb, bq_sb, qhT),
                    ("k", kT, wk_sb, bk_sb, khT),
                ):
                    x_sb = ph0.tile([128, nech, s_len], FP, tag="x_in")
                    nc.sync.dma_start(out=x_sb[:],
                                      in_=src.rearrange("(c p) s -> p c s", p=128))
                    for dg in range(2):          # head-pairs (0,1) and (2,3)
                        for sc in range((s_len + 511) // 512):
                            s0, s1 = sc * 512, min((sc + 1) * 512, s_len)
                            ps = ph0ps.tile([64, 512], FP, tag=f"projps_{name}")
                            for c in range(nech):
                                nc.tensor.matmul(ps[:, 0:s1 - s0],
                                                 lhsT=wsb[:, c, dg * 64:(dg + 1) * 64],
                                                 rhs=x_sb[:, c, s0:s1],
                                                 start=(c == 0), stop=(c == nech - 1))
                            nc.scalar.activation(out=dst[:, dg, s0:s1],
                                                 in_=ps[:, 0:s1 - s0], func=AF.Identity,
                                                 bias=bias[:, dg:dg + 1])

                # vh: natural [s, d] orientation + bias row + bf16 cast
                xv_sb = ph0.tile([128, nech, s_len], FP, tag="x_in")
                nc.sync.dma_start(out=xv_sb[:],
                                  in_=vT.rearrange("(c p) s -> p c s", p=128))
                for sb in range(nqb):
                    ps = ph0ps.tile([128, 128], FP, tag="vps")
                    for c in range(nech):
                        nc.tensor.matmul(ps[:], lhsT=xv_sb[:, c, sb * 128:(sb + 1) * 128],
                                         rhs=wv_sb[:, c, :], start=(c == 0), stop=False)
                    nc.tensor.matmul(ps[:], lhsT=ones1[:], rhs=bv_sb[:],
                                     start=False, stop=True)
                    for h in range(HG):
                        nc.scalar.activation(out=vh1[:, sb, h, 0:32],
                                             in_=ps[:, h * 32:(h + 1) * 32], func=AF.Copy)

            # ---- attention loop ----
            with (
                tc.tile_pool(name="att", bufs=1) as att,
                tc.tile_pool(name="att2", bufs=2) as att2,
                tc.tile_pool(name="att3", bufs=3) as att3,
                tc.tile_pool(name="ps_sm", bufs=1, space="PSUM") as ps_sm,
                tc.tile_pool(name="ps_lg", bufs=1, space="PSUM") as ps_lg,
                tc.tile_pool(name="ps_av", bufs=1, space="PSUM") as ps_av,
                tc.tile_pool(name="ps_op", bufs=1, space="PSUM") as ps_op,
            ):
                # pair small and large extents so the two PSUM score buffers
                # (2 + 4 banks) let two tiles pipeline at once
                half = (nqb + 1) // 2
                pairs = [(lo, nqb - 1 - lo) for lo in range(half)]

                def attn_tile(kq, h, concat):
                    nb = kq + 1            # causal extent in 128-key blocks
                    N = nb * 128
                    small = kq < half
                    pool = ps_sm if small else ps_lg

                    def qk_scores():
                        # QK^T chunks of <=512 moving columns + diagonal mask
                        s_ps = pool.tile([128, half * 128 if small else nqb * 128],
                                         FP, tag="s_sm" if small else "s_lg")
                        hp, hb = (h % 2) * 32, h // 2
                        nchunk = (N + 511) // 512
                        for c in range(nchunk):
                            c0, c1 = c * 512, min((c + 1) * 512, N)
                            nc.tensor.matmul(
                                s_ps[:, c0:c1],
                                lhsT=qhT[hp:hp + 32, hb, kq * 128:(kq + 1) * 128],
                                rhs=khT[hp:hp + 32, hb, c0:c1],
                                start=True, stop=True, skip_group_check=True)
                        nc.tensor.matmul(s_ps[:, N - 128:N], lhsT=ident_b[:],
                                         rhs=triu_neg[:], start=False, stop=True,
                                         skip_group_check=True)
                        return s_ps

                    # scores are cheap to recompute (bf16 QK): materialize them
                    # twice so the PSUM slot is held only across one consumer
                    # (exp1, then the s2 product) instead of the whole chain
                    s_ps = qk_scores()

                    # softmax #1 numerator (no max shift needed, |s| is small)
                    e = att3.tile([128, s_len], BF, tag="e")
                    nc.scalar.activation(out=e[:, :N], in_=s_ps[:, :N], func=AF.Exp)

                    # suffix sums: tail[j] = sum_{j'>=j} e[j'] (reversed scan);
                    # tail[0] is the softmax denominator, tail[j+1] the
                    # exclusive tail the decay term needs
                    tail = att3.tile([128, s_len + 2], BF, tag="tail")
                    nc.gpsimd.memset(tail[:, N:N + 1], 0.0)
                    nc.vector.tensor_tensor_scan(
                        out=tail[:, 0:N][:, ::-1], data0=e[:, 0:N][:, ::-1],
                        data1=e[:, 0:N][:, ::-1], initial=0.0,
                        op0=OP.add, op1=OP.bypass)

                    # log-domain decay: dist = exp(0.5*(ln tail + ln pos
                    # + ln gamma^2 - ln sigma)); Ln+Exp share one ACT table
                    # set (Sqrt doesn't fit beside Exp); tail or pos = +0
                    # gives -inf -> dist=0 -> te=1 exactly
                    lnt = att3.tile([128, s_len + 2], FP, tag="lnt")
                    nc.scalar.activation(out=lnt[:, 0:N + 1], in_=tail[:, 0:N + 1],
                                         func=AF.Ln)
                    # c_h = ln gamma_h^2 - ln sigma1   [128,1]
                    ch = att3.tile([128, 1], FP, tag="ch")
                    nc.vector.tensor_scalar(out=ch[:], in0=lnt[:, 0:1],
                                            scalar1=-1.0,
                                            scalar2=lngsq_sb[:, h:h + 1],
                                            op0=OP.mult, op1=OP.add)
                    # u = ln(tail_excl) + c_h + ln(pos), in place (cols 1..N)
                    nc.vector.scalar_tensor_tensor(
                        out=lnt[:, 1:N + 1], in0=lnt[:, 1:N + 1], scalar=ch[:],
                        in1=lnposM[:, 127 + 128 * kq::-1],
                        op0=OP.add, op1=OP.add)
                    nc.scalar.activation(out=lnt[:, 1:N + 1], in_=lnt[:, 1:N + 1],
                                         func=AF.Exp, scale=0.5)
                    nc.scalar.activation(out=lnt[:, 1:N + 1], in_=lnt[:, 1:N + 1],
                                         func=AF.Exp, scale=-1.0)

                    # s2 = max(te, 1e-5) * s   (masked lanes stay ~ -1e30)
                    s_ps2 = qk_scores()
                    s2 = att3.tile([128, s_len], FP, tag="s2")
                    nc.vector.scalar_tensor_tensor(
                        out=s2[:, :N], in0=lnt[:, 1:N + 1], scalar=1e-5,
                        in1=s_ps2[:, :N], op0=OP.max, op1=OP.mult)
                    # softmax #2 numerator
                    e2 = att3.tile([128, s_len], BF, tag="e2")
                    nc.scalar.activation(out=e2[:, :N], in_=s2[:, :N], func=AF.Exp)

                    e2t = att3.tile([128, nqb, 128], BF, tag="e2t")
                    nc.sync.dma_start_transpose(out=e2t[:, 0:nb, :], in_=e2[:, :N])
                    av = ps_av.tile([128, 64], FP, tag="av")
                    for c in range(nb):
                        nc.tensor.matmul(av[:, 0:33], lhsT=e2t[:, c, :],
                                         rhs=vh1[:, c, h, :],
                                         start=(c == 0), stop=(c == nb - 1))
                    rec2 = att3.tile([128, 1], FP, tag="rec2")
                    nc.vector.reciprocal(out=rec2[:], in_=av[:, 32:33])
                    nc.vector.tensor_scalar(
                        out=concat[:, h * 32:(h + 1) * 32], in0=av[:, 0:32],
                        scalar1=rec2[:], scalar2=None, op0=OP.mult)

                def out_proj(kq, concat):
                    trp = ps_op.tile([128, 128], FP, tag="trop")
                    nc.tensor.transpose(out=trp[:], in_=concat[:], identity=ident_f[:])
                    concatT = att2.tile([128, 128], FP, tag="concatT")
                    nc.scalar.activation(out=concatT[:], in_=trp[:], func=AF.Copy)
                    op = ps_op.tile([128, 256], FP, tag="trop")
                    nc.tensor.matmul(op[:], lhsT=concatT[:], rhs=wo_sb[:],
                                     start=True, stop=True)
                    ostg = att2.tile([128, 256], FP, tag="ostg")
                    nc.scalar.activation(out=ostg[:], in_=op[:], func=AF.Copy)
                    nc.sync.dma_start(out=out_part[kq * 128:(kq + 1) * 128, :],
                                      in_=ostg[:])

                for ksm, klg in pairs:
                    concat_s = att2.tile([128, 128], FP, tag="concat_s")
                    if klg != ksm:
                        concat_l = att2.tile([128, 128], FP, tag="concat_l")
                    for h in range(HG):
                        attn_tile(ksm, h, concat_s)
                        if klg != ksm:
                            attn_tile(klg, h, concat_l)
                    out_proj(ksm, concat_s)
                    if klg != ksm:
                        out_proj(klg, concat_l)
    return nc


# ---------------------------------------------------------------------------
# host side
# ---------------------------------------------------------------------------

def _softplus(x):
    return np.logaddexp(0.0, x)


def _make_in_maps(q, k, v, Wq, bq, Wk, bk, Wv, bv, Wo, gammas, s_len=S):
    scale = 1.0 / np.sqrt(np.float32(DK))
    g = -_softplus(gammas.reshape(H).astype(np.float64)).astype(np.float32)
    in_maps = []
    for core in range(NCORES):
        b, grp = core // 2, core % 2
        hsel = slice(grp * HG * DK, (grp + 1) * HG * DK)   # rows of W, dims of proj
        gam = g[grp * HG:(grp + 1) * HG]
        in_maps.append({
            "qT": np.ascontiguousarray(q[b].T.astype(np.float32)),
            "kT": np.ascontiguousarray(k[b].T.astype(np.float32)),
            "vT": np.ascontiguousarray(v[b].T.astype(np.float32)),
            "wqT": np.ascontiguousarray((Wq[hsel, :] * scale).T.astype(np.float32)),
            "wkT": np.ascontiguousarray(Wk[hsel, :].T.astype(np.float32)),
            "wvT": np.ascontiguousarray(Wv[hsel, :].T.astype(np.float32)),
            "woT": np.ascontiguousarray(Wo[:, hsel].T.astype(np.float32)),
            "bqs": np.ascontiguousarray(
                (bq[hsel] * scale).astype(np.float32).reshape(2, 64).T),
            "bks": np.ascontiguousarray(
                bk[hsel].astype(np.float32).reshape(2, 64).T),
            "bvrow": bv[hsel].astype(np.float32).reshape(1, D),
            "lngsq": np.broadcast_to(
                (2.0 * np.log(-gam)).astype(np.float32), (128, HG)).copy(),
        })
    return in_maps


_NC_CACHE = {}


def _get_nc(s_len=S):
    if s_len not in _NC_CACHE:
        nc = build_nc(s_len)
        nc.finalize()      # Bacc pipeline: wait splitting, reg alloc, DCE
        _NC_CACHE[s_len] = nc
    return _NC_CACHE[s_len]


def kernel(q, k, v, mask, Wq, bq, Wk, bk, Wv, bv, Wo, bo, gammas):
    """Full-input, full-output entry point.  `mask` is the causal mask the
    reference builds; the kernel hardcodes causality."""
    from concourse.bass_utils import run_bass_kernel_spmd

    q, k, v = (np.asarray(a, np.float32) for a in (q, k, v))
    in_maps = _make_in_maps(q, k, v, np.asarray(Wq), np.asarray(bq),
                            np.asarray(Wk), np.asarray(bk), np.asarray(Wv),
                            np.asarray(bv), np.asarray(Wo),
                            np.asarray(gammas))
    nc = _get_nc(S)
    res = run_bass_kernel_spmd(nc, in_maps, core_ids=list(range(NCORES)))
    parts = [res.results[c]["out_part"] for c in range(NCORES)]
    out = np.empty((B, S, E), np.float32)
    bo = np.asarray(bo, np.float32)
    for b in range(B):
        out[b] = parts[2 * b] + parts[2 * b + 1] + bo[None, :]
    return out


# revision 26
# speedup vs baseline: 1.4569x; 1.0063x over previous
"""Trainium2 Bass kernel for the AKT (attention-with-distance-decay) problem.

Reference math (per batch b, head h, dk=32, S=2048, E=256):
    qh, kh, vh = per-head projections of q,k,v
    s  = qh @ kh^T / sqrt(dk)                    (causal-masked)
    p  = softmax(s)                              (softmax #1)
    tail[j] = sum_{j'>j} p[j']                   (1 - cumsum)
    dist = sqrt(clip(tail * (i-j), 0))
    te   = clip(exp(-softplus(gamma_h) * dist), 1e-5, 1e5)
    attn = softmax(where(mask, s*te, -inf))      (softmax #2)
    out  = (attn @ vh)  -> concat heads -> @ Wo^T + bo

Sharding: 8 cores = (batch b = core//2) x (head-group g = core%2, 4 heads each).
Every core runs the identical graph (SPMD); per-core inputs differ.  Each core
emits a partial output (its 4 heads' contribution through Wo); the host adds
the two partials per batch plus bo.

Device-side structure per core:
  - host pre-transposes q/k/v to [E, S] so projections contract over e on
    the partition dim; Wq and bq are pre-scaled by 1/sqrt(dk).
  - qh^T, kh^T stored [128(4h x 32d), S]; vh stored [S, 4h, 33] bf16 with a
    ones column so the AV matmul also yields the softmax-#2 denominator.
  - causal q-block loop (128 queries, extent = (k+1) key-blocks); the
    diagonal block is masked by accumulating ident^T @ triu(-1e30) onto the
    QK PSUM scores.
  - softmax #1 skips the max-subtraction (scores are O(5), fp32 exp is safe);
    the key-axis cumsum is a REVERSED tensor_tensor_scan giving the exact
    suffix-sum (no 1-x cancellation); its col 0 is the softmax denominator.
  - te = exp(-sqrt(gamma^2 * tail * pos / sigma)): gamma^2/sigma ride the
    scalar slot of one scalar_tensor_tensor; sqrt+exp fused across 4 heads.
  - softmax #2: e2 = exp(s * clip(te)) directly (no max, masked lanes are
    exp(-1e30)=0); Sigma2 via the ones column; normalization folded into a
    per-partition tensor_scalar on the AV output.
  - e2 (bf16) transposed for AV by the DMA xbar (sync engine), not PE.
"""

import os
import sys

for _p in ("/opt/trn_rl_repo", "/root/.axon_site/_ro/trn_rl_repo"):
    if os.path.isdir(_p) and _p not in sys.path:
        sys.path.insert(0, _p)

import numpy as np

import concourse.bacc as bacc
import concourse.bass as bass
import concourse.mybir as mybir
from concourse.tile import TileContext

B, S, E, H = 4, 2048, 256, 8
DK = E // H          # 32
HG = 4               # heads per core
D = HG * DK          # 128, per-core projected width
NCORES = 8

FP = mybir.dt.float32
BF = mybir.dt.bfloat16
AF = mybir.ActivationFunctionType
OP = mybir.AluOpType
NEG = -1e30


class _AktBacc(bacc.Bacc):
    """Bacc whose activation-table placement only considers the one set
    covering every ACT function this kernel uses (Exp, Ln, Identity, Copy).
    The default first-match policy alternates exp_and_others with a
    Ln-capable set, reloading the 2.7us ACT tables per tile."""

    _ACT_SET = "natural_log_exp_and_others"

    def insert_act_table_loads(self):
        import concourse.mybir as _mb
        from concourse.hw_specs import get_activation_tables
        has_activation = any(
            isinstance(i, _mb.InstActivation)
            for b in self.main_func.blocks
            for i in b.instructions
        )
        if not has_activation:
            return
        # positions must stay canonical (act_func_set_id indexes this list)
        tables = [
            (nm, fs if nm == self._ACT_SET else set())
            for nm, fs in get_activation_tables(self.m.arch).items()
        ]
        import bass_rust as _br
        _br.insert_act_table_loads(self, tables)


def build_nc(s_len=S, qk_f32r=False):
    """Build the single-core SPMD graph.  s_len parametrizes the sequence
    length for small-scale simulation tests (must be a multiple of 128)."""
    nqb = s_len // 128           # number of 128-query blocks
    nech = E // 128              # e-chunks (2)

    nc = _AktBacc()
    qT = nc.declare_dram_parameter("qT", [E, s_len], FP, isOutput=False)
    kT = nc.declare_dram_parameter("kT", [E, s_len], FP, isOutput=False)
    vT = nc.declare_dram_parameter("vT", [E, s_len], FP, isOutput=False)
    wqT = nc.declare_dram_parameter("wqT", [E, D], FP, isOutput=False)
    wkT = nc.declare_dram_parameter("wkT", [E, D], FP, isOutput=False)
    wvT = nc.declare_dram_parameter("wvT", [E, D], FP, isOutput=False)
    woT = nc.declare_dram_parameter("woT", [D, E], FP, isOutput=False)
    bqs = nc.declare_dram_parameter("bqs", [64, 2], FP, isOutput=False)
    bks = nc.declare_dram_parameter("bks", [64, 2], FP, isOutput=False)
    bvrow = nc.declare_dram_parameter("bvrow", [1, D], FP, isOutput=False)
    lngsq = nc.declare_dram_parameter("lngsq", [128, HG], FP, isOutput=False)
    out_part = nc.declare_dram_parameter("out_part", [s_len, E], FP, isOutput=True)

    qk_dt = mybir.dt.float32r if qk_f32r else FP

    with TileContext(nc) as tc:
        with (
            tc.tile_pool(name="consts", bufs=1) as consts,
            tc.tile_pool(name="persist", bufs=1) as persist,
        ):
            # ---- constants ----
            ident_f = consts.tile([128, 128], FP)
            nc.vector.memset(ident_f[:], 1.0)
            nc.gpsimd.affine_select(out=ident_f[:], in_=ident_f[:],
                                    compare_op=OP.is_equal, fill=0.0,
                                    base=0, pattern=[[-1, 128]], channel_multiplier=1)
            ident_b = consts.tile([128, 128], BF)
            nc.vector.tensor_copy(out=ident_b[:], in_=ident_f[:])
            # strict upper triangle = NEG, else 0 (diagonal-block causal mask)
            triu_neg = consts.tile([128, 128], BF)
            nc.gpsimd.memset(triu_neg[:], 0.0)
            nc.gpsimd.affine_select(out=triu_neg[:], in_=triu_neg[:],
                                    compare_op=OP.is_ge, fill=NEG,
                                    base=0, pattern=[[-1, 128]], channel_multiplier=1)
            ones1 = consts.tile([1, 128], FP)
            nc.vector.memset(ones1[:], 1.0)

            lngsq_sb = consts.tile([128, HG], FP)
            nc.sync.dma_start(out=lngsq_sb[:], in_=lngsq[:])
            bq_sb = consts.tile([64, 2], FP)
            nc.sync.dma_start(out=bq_sb[:], in_=bqs[:])
            bk_sb = consts.tile([64, 2], FP)
            nc.sync.dma_start(out=bk_sb[:], in_=bks[:])
            bv_sb = consts.tile([1, D], FP)
            nc.sync.dma_start(out=bv_sb[:], in_=bvrow[:])
            wo_sb = consts.tile([D, E], FP)
            nc.sync.dma_start(out=wo_sb[:], in_=woT[:])

            # master ln(pos) table: lnpos_k[:, j] = M[:, 127 + 128k - j]
            # (a reversed AP view), M[r, c] = ln(r + c - 127), -inf at pos<=0
            lnposM = persist.tile([128, s_len], FP)
            nc.gpsimd.iota(lnposM[:], pattern=[[1, s_len]], base=-127,
                           channel_multiplier=1,
                           allow_small_or_imprecise_dtypes=True)
            nc.gpsimd.affine_select(out=lnposM[:], in_=lnposM[:],
                                    compare_op=OP.is_ge, fill=0.0,
                                    base=-127, pattern=[[1, s_len]],
                                    channel_multiplier=1)
            nc.scalar.activation(out=lnposM[:], in_=lnposM[:], func=AF.Ln)

            # ---- persistent activations ----
            # head h lives at partitions (h%2)*32..+32, free-block h//2
            # (PE operands may only start at partition 0/32/64)
            qhT = persist.tile([64, 2, s_len], BF)
            khT = persist.tile([64, 2, s_len], BF)
            vh1 = persist.tile([128, nqb, HG, 33], BF)  # [s-part, s-blk, h, 32d+1]
            nc.vector.memset(vh1[:, :, :, 32:33], 1.0)

            # ---- phase 0: projections ----
            with (
                tc.tile_pool(name="ph0", bufs=2) as ph0,
                tc.tile_pool(name="ph0w", bufs=1) as ph0w,
                tc.tile_pool(name="ph0ps", bufs=2, space="PSUM") as ph0ps,
            ):
                wq_sb = ph0w.tile([128, nech, D], FP)
                wk_sb = ph0w.tile([128, nech, D], FP)
                wv_sb = ph0w.tile([128, nech, D], FP)
                nc.sync.dma_start(out=wq_sb[:], in_=wqT.rearrange("(c p) d -> p c d", p=128))
                nc.sync.dma_start(out=wk_sb[:], in_=wkT.rearrange("(c p) d -> p c d", p=128))
                nc.sync.dma_start(out=wv_sb[:], in_=wvT.rearrange("(c p) d -> p c d", p=128))

                for name, src, wsb, bias, dst in (
                    ("q", qT, wq_sb, bq_sb, qhT),
                    ("k", kT, wk_sb, bk_sb, khT),
                ):
                    x_sb = ph0.tile([128, nech, s_len], FP, tag="x_in")
                    nc.sync.dma_start(out=x_sb[:],
                                      in_=src.rearrange("(c p) s -> p c s", p=128))
                    for dg in range(2):          # head-pairs (0,1) and (2,3)
                        for sc in range((s_len + 511) // 512):
                            s0, s1 = sc * 512, min((sc + 1) * 512, s_len)
                            ps = ph0ps.tile([64, 512], FP, tag=f"projps_{name}")
                            for c in range(nech):
                                nc.tensor.matmul(ps[:, 0:s1 - s0],
                                                 lhsT=wsb[:, c, dg * 64:(dg + 1) * 64],
                                                 rhs=x_sb[:, c, s0:s1],
                                                 start=(c == 0), stop=(c == nech - 1))
                            nc.scalar.activation(out=dst[:, dg, s0:s1],
                                                 in_=ps[:, 0:s1 - s0], func=AF.Identity,
                                                 bias=bias[:, dg:dg + 1])

                # vh: natural [s, d] orientation + bias row + bf16 cast
                xv_sb = ph0.tile([128, nech, s_len], FP, tag="x_in")
                nc.sync.dma_start(out=xv_sb[:],
                                  in_=vT.rearrange("(c p) s -> p c s", p=128))
                for sb in range(nqb):
                    ps = ph0ps.tile([128, 128], FP, tag="vps")
                    for c in range(nech):
                        nc.tensor.matmul(ps[:], lhsT=xv_sb[:, c, sb * 128:(sb + 1) * 128],
                                         rhs=wv_sb[:, c, :], start=(c == 0), stop=False)
                    nc.tensor.matmul(ps[:], lhsT=ones1[:], rhs=bv_sb[:],
                                     start=False, stop=True)
                    for h in range(HG):
                        nc.scalar.activation(out=vh1[:, sb, h, 0:32],
                                             in_=ps[:, h * 32:(h + 1) * 32], func=AF.Copy)

            # ---- attention loop: 3-stage software pipeline ----
            # wave = the 4 heads of one q-block; waves alternate small/large
            # extents so stage A (first QK+exp) and stage C (second QK, AV)
            # of different waves use different PSUM pools.  Stage-major
            # emission keeps each engine's in-order queue free of
            # head-of-line stalls: while ACT runs wave w's exp1s, DVE runs
            # wave w-1's scans and PE runs wave w-2's AV matmuls.
            with (
                tc.tile_pool(name="att2", bufs=2) as att2,
                tc.tile_pool(name="atte", bufs=6) as atte,
                tc.tile_pool(name="att4", bufs=4) as att4,
                tc.tile_pool(name="att8", bufs=8) as att8,
                tc.tile_pool(name="ps_sm", bufs=1, space="PSUM") as ps_sm,
                tc.tile_pool(name="ps_lg", bufs=1, space="PSUM") as ps_lg,
                tc.tile_pool(name="ps_av", bufs=1, space="PSUM") as ps_av,
                tc.tile_pool(name="ps_op", bufs=1, space="PSUM") as ps_op,
            ):
                half = (nqb + 1) // 2
                waves = []
                lo, hi = 0, nqb - 1
                while lo <= hi:
                    waves.append(lo)
                    if hi != lo:
                        waves.append(hi)
                    lo += 1
                    hi -= 1

                def qk_scores(kq, h):
                    # QK^T chunks of <=512 moving columns + diagonal mask
                    N = (kq + 1) * 128
                    small = kq < half
                    pool = ps_sm if small else ps_lg
                    s_ps = pool.tile([128, half * 128 if small else nqb * 128],
                                     FP, tag="s_sm" if small else "s_lg")
                    hp, hb = (h % 2) * 32, h // 2
                    nchunk = (N + 511) // 512
                    for c in range(nchunk):
                        c0, c1 = c * 512, min((c + 1) * 512, N)
                        nc.tensor.matmul(
                            s_ps[:, c0:c1],
                            lhsT=qhT[hp:hp + 32, hb, kq * 128:(kq + 1) * 128],
                            rhs=khT[hp:hp + 32, hb, c0:c1],
                            start=True, stop=True, skip_group_check=True)
                    nc.tensor.matmul(s_ps[:, N - 128:N], lhsT=ident_b[:],
                                     rhs=triu_neg[:], start=False, stop=True,
                                     skip_group_check=True)
                    return s_ps

                def stage_a(kq):
                    """first scores + softmax-#1 numerators for 4 heads"""
                    N = (kq + 1) * 128
                    es = []
                    for h in range(HG):
                        s_ps = qk_scores(kq, h)
                        e = atte.tile([128, s_len], BF, tag="e")
                        nc.scalar.activation(out=e[:, :N], in_=s_ps[:, :N],
                                             func=AF.Exp)
                        es.append(e)
                    return es

                def stage_b(kq, es):
                    """suffix-sum scan + log-domain decay -> te (bf16)"""
                    N = (kq + 1) * 128
                    tails, lnts, tes = [], [], []
                    for h in range(HG):
                        tail = att4.tile([128, s_len + 2], BF, tag="tail")
                        nc.gpsimd.memset(tail[:, N:N + 1], 0.0)
                        nc.vector.tensor_tensor_scan(
                            out=tail[:, 0:N][:, ::-1], data0=es[h][:, 0:N][:, ::-1],
                            data1=es[h][:, 0:N][:, ::-1], initial=0.0,
                            op0=OP.add, op1=OP.bypass)
                        tails.append(tail)
                    for h in range(HG):
                        # dist = exp(0.5*(ln tail + ln pos + ln gamma^2
                        # - ln sigma)); Ln+Exp share one ACT table set;
                        # tail or pos = +0 gives -inf -> dist=0 -> te=1
                        lnt = att4.tile([128, s_len + 2], FP, tag="lnt")
                        nc.scalar.activation(out=lnt[:, 0:N + 1],
                                             in_=tails[h][:, 0:N + 1], func=AF.Ln)
                        lnts.append(lnt)
                    chs = []
                    for h in range(HG):
                        ch = att4.tile([128, 1], FP, tag="ch")
                        nc.vector.tensor_scalar(out=ch[:], in0=lnts[h][:, 0:1],
                                                scalar1=-1.0,
                                                scalar2=lngsq_sb[:, h:h + 1],
                                                op0=OP.mult, op1=OP.add)
                        chs.append(ch)
                    for h in range(HG):
                        nc.vector.scalar_tensor_tensor(
                            out=lnts[h][:, 1:N + 1], in0=lnts[h][:, 1:N + 1],
                            scalar=chs[h], in1=lnposM[:, 127 + 128 * kq::-1],
                            op0=OP.add, op1=OP.add)
                    for h in range(HG):
                        nc.scalar.activation(out=lnts[h][:, 1:N + 1],
                                             in_=lnts[h][:, 1:N + 1],
                                             func=AF.Exp, scale=0.5)
                    for h in range(HG):
                        te = att8.tile([128, s_len], BF, tag="te")
                        nc.scalar.activation(out=te[:, :N],
                                             in_=lnts[h][:, 1:N + 1],
                                             func=AF.Exp, scale=-1.0)
                        tes.append(te)
                    return tes

                def stage_c(kq, tes):
                    """second scores, softmax #2, AV, output projection"""
                    N = (kq + 1) * 128
                    nb = kq + 1
                    concat = att2.tile([128, 128], FP, tag="concat")
                    e2s = []
                    for h in range(HG):
                        s_ps2 = qk_scores(kq, h)
                        # s2 = max(te, 1e-5) * s  (masked lanes stay ~ -1e30)
                        s2 = att2.tile([128, s_len], FP, tag="s2")
                        nc.vector.scalar_tensor_tensor(
                            out=s2[:, :N], in0=tes[h][:, :N], scalar=1e-5,
                            in1=s_ps2[:, :N], op0=OP.max, op1=OP.mult)
                        e2 = att2.tile([128, s_len], BF, tag="e2")
                        nc.scalar.activation(out=e2[:, :N], in_=s2[:, :N],
                                             func=AF.Exp)
                        e2s.append(e2)
                    e2ts = []
                    for h in range(HG):
                        e2t = att2.tile([128, nqb, 128], BF, tag="e2t")
                        nc.sync.dma_start_transpose(out=e2t[:, 0:nb, :],
                                                    in_=e2s[h][:, :N])
                        e2ts.append(e2t)
                    for h in range(HG):
                        av = ps_av.tile([128, 64], FP, tag="av")
                        for c in range(nb):
                            nc.tensor.matmul(av[:, 0:33], lhsT=e2ts[h][:, c, :],
                                             rhs=vh1[:, c, h, :],
                                             start=(c == 0), stop=(c == nb - 1))
                        rec2 = att4.tile([128, 1], FP, tag="rec2")
                        nc.vector.reciprocal(out=rec2[:], in_=av[:, 32:33])
                        nc.vector.tensor_scalar(
                            out=concat[:, h * 32:(h + 1) * 32], in0=av[:, 0:32],
                            scalar1=rec2[:], scalar2=None, op0=OP.mult)
                    # output projection for this q-block
                    trp = ps_op.tile([128, 128], FP, tag="trop")
                    nc.tensor.transpose(out=trp[:], in_=concat[:],
                                        identity=ident_f[:])
                    concatT = att2.tile([128, 128], FP, tag="concatT")
                    nc.scalar.activation(out=concatT[:], in_=trp[:], func=AF.Copy)
                    op = ps_op.tile([128, 256], FP, tag="trop")
                    nc.tensor.matmul(op[:], lhsT=concatT[:], rhs=wo_sb[:],
                                     start=True, stop=True)
                    ostg = att2.tile([128, 256], FP, tag="ostg")
                    nc.scalar.activation(out=ostg[:], in_=op[:], func=AF.Copy)
                    nc.sync.dma_start(out=out_part[kq * 128:(kq + 1) * 128, :],
                                      in_=ostg[:])

                state = {}
                for i in range(len(waves) + 2):
                    if i < len(waves):
                        state[i] = (waves[i], stage_a(waves[i]))
                    if 0 <= i - 1 < len(waves):
                        kq, es = state[i - 1]
                        state[i - 1] = (kq, stage_b(kq, es))
                    if 0 <= i - 2 < len(waves):
                        kq, tes = state.pop(i - 2)
                        stage_c(kq, tes)
    return nc


# ---------------------------------------------------------------------------
# host side
# ---------------------------------------------------------------------------

def _softplus(x):
    return np.logaddexp(0.0, x)


def _make_in_maps(q, k, v, Wq, bq, Wk, bk, Wv, bv, Wo, gammas, s_len=S):
    scale = 1.0 / np.sqrt(np.float32(DK))
    g = -_softplus(gammas.reshape(H).astype(np.float64)).astype(np.float32)
    in_maps = []
    for core in range(NCORES):
        b, grp = core // 2, core % 2
        hsel = slice(grp * HG * DK, (grp + 1) * HG * DK)   # rows of W, dims of proj
        gam = g[grp * HG:(grp + 1) * HG]
        in_maps.append({
            "qT": np.ascontiguousarray(q[b].T.astype(np.float32)),
            "kT": np.ascontiguousarray(k[b].T.astype(np.float32)),
            "vT": np.ascontiguousarray(v[b].T.astype(np.float32)),
            "wqT": np.ascontiguousarray((Wq[hsel, :] * scale).T.astype(np.float32)),
            "wkT": np.ascontiguousarray(Wk[hsel, :].T.astype(np.float32)),
            "wvT": np.ascontiguousarray(Wv[hsel, :].T.astype(np.float32)),
            "woT": np.ascontiguousarray(Wo[:, hsel].T.astype(np.float32)),
            "bqs": np.ascontiguousarray(
                (bq[hsel] * scale).astype(np.float32).reshape(2, 64).T),
            "bks": np.ascontiguousarray(
                bk[hsel].astype(np.float32).reshape(2, 64).T),
            "bvrow": bv[hsel].astype(np.float32).reshape(1, D),
            "lngsq": np.broadcast_to(
                (2.0 * np.log(-gam)).astype(np.float32), (128, HG)).copy(),
        })
    return in_maps


_NC_CACHE = {}


def _get_nc(s_len=S):
    if s_len not in _NC_CACHE:
        nc = build_nc(s_len)
        nc.finalize()      # Bacc pipeline: wait splitting, reg alloc, DCE
        _NC_CACHE[s_len] = nc
    return _NC_CACHE[s_len]


def kernel(q, k, v, mask, Wq, bq, Wk, bk, Wv, bv, Wo, bo, gammas):
    """Full-input, full-output entry point.  `mask` is the causal mask the
    reference builds; the kernel hardcodes causality."""
    from concourse.bass_utils import run_bass_kernel_spmd

    q, k, v = (np.asarray(a, np.float32) for a in (q, k, v))
    in_maps = _make_in_maps(q, k, v, np.asarray(Wq), np.asarray(bq),
                            np.asarray(Wk), np.asarray(bk), np.asarray(Wv),
                            np.asarray(bv), np.asarray(Wo),
                            np.asarray(gammas))
    nc = _get_nc(S)
    res = run_bass_kernel_spmd(nc, in_maps, core_ids=list(range(NCORES)))
    parts = [res.results[c]["out_part"] for c in range(NCORES)]
    out = np.empty((B, S, E), np.float32)
    bo = np.asarray(bo, np.float32)
    for b in range(B):
        out[b] = parts[2 * b] + parts[2 * b + 1] + bo[None, :]
    return out
